# revision 32
# baseline (speedup 1.0000x reference)
"""Causal self-attention with ALiBi — Trainium2 Bass kernel, 8-core SPMD.

Problem: y = softmax(mask(q k^T / sqrt(hd) + alibi)) v, with q/kv/o projections.
B=2, T=2048, C=1024, NH=16, HD=64.

Sharding: core c handles batch b = c//4 and heads [4*(c%4), 4*(c%4)+4).
Projections are tensor-parallel over heads; each core emits a partial
o-projection (its 256 channels' contribution); the host sums the 4 partials
per batch (plus the bias terms, which are folded in analytically).

v3 design notes:
- The full ALiBi term rides inside the QK^T matmul via FOUR augmentation row
  pairs: kaug row 64 = slope (pairs with qaug row 64 = -i), and kaug rows
  65..67 = slope*(j%16), slope*16*((j//16)%16), slope*256*(j//256) pairing
  with qaug ones-rows. Each key-side value has an integer numerator <= 240 so
  it is EXACT in bf16; the fp32 psum sum reconstructs slope*j exactly.
  Query-side (-i) rounding cancels per-query in softmax. The Exp activation
  then needs no per-key-tile bias, so one exp covers a GROUP of key tiles
  packed back-to-back in one single-bank [128,512] psum tile.
- ALiBi windows tightened to theta=10 e-foldings (host-checked: ~1e-5 err).
- Score tiles are single-bank with bufs=4: the QK stream runs 2 groups ahead
  of the mask/exp/AV chain so the PE never blocks on ACT/DVE.
- k-projection bias dropped (a key-side bias cancels exactly in softmax).
- All projection psums are single-bank halves (double-buffered 8-bank psum:
  4 score + 2 y + 2 proj).
- Engine placement: exp + half the q/k copies on ACT, normalize + the other
  copies on DVE, tri-mask + v-scatter + memsets on Pool.
- Attention is a generator; projection half-chunks and v/o tiles interleave
  as PE filler between attention groups.
- o-projection rows 0..1023 DMA straight from psum to DRAM in fp32 (no
  engine copy); the tail rows 1024..2047 (engines idle by then) go through
  engine copies to bf16.
"""

import numpy as np
import ml_dtypes

B, T, C = 2, 2048, 1024
NH, HD = 16, 64
NCORES = 8
NHL = 4          # heads per core
W = 1024         # query superchunk width
NQS = T // W     # superchunks
KT = T // 128    # key tiles
CT = 2           # channel tiles for q/k projections (256 channels / 128)
KIN = C // 128   # contraction tiles for projections
TT = T // 128    # token tiles
NEG = -1.0e30
THETA = 8        # ALiBi window e-foldings
DWIN = [(THETA * 16 + hl) // (hl + 1) for hl in range(NHL)]
QKSCALE = 64.0   # fp8 weight pre-scale for the q/k projections

BF16 = ml_dtypes.bfloat16

_CACHE = {}


def _build_nc():
    import concourse.mybir as mybir
    import concourse.tile as tile
    from concourse import bacc

    f32 = mybir.dt.float32
    bf16 = mybir.dt.bfloat16
    f8 = mybir.dt.float8e4
    Exp = mybir.ActivationFunctionType.Exp
    Ident = mybir.ActivationFunctionType.Identity
    DR = mybir.MatmulPerfMode.DoubleRow

    nc = bacc.Bacc("TRN2", target_bir_lowering=False, debug=False,
                   enable_asserts=False, num_devices=NCORES)

    xt_d = nc.dram_tensor("xt", [C, T], bf16, kind="ExternalInput")
    xq8_d = nc.dram_tensor("xq8", [C, T], f8, kind="ExternalInput")
    wq_d = nc.dram_tensor("wq", [C, 256], f8, kind="ExternalInput")
    wk_d = nc.dram_tensor("wk", [C, 256], f8, kind="ExternalInput")
    wv_d = nc.dram_tensor("wv", [C, 256], bf16, kind="ExternalInput")
    wo_d = nc.dram_tensor("wo", [256, C], bf16, kind="ExternalInput")
    qb_d = nc.dram_tensor("qb", [128, CT], f32, kind="ExternalInput")
    qrow_d = nc.dram_tensor("qrow", [4, T], bf16, kind="ExternalInput")
    krows_d = nc.dram_tensor("krows", [NHL * 4, T], bf16, kind="ExternalInput")
    tri_d = nc.dram_tensor("tri", [128, 128], f32, kind="ExternalInput")
    out_d = nc.dram_tensor("o_part", [T, C], bf16, kind="ExternalOutput")

    with tile.TileContext(nc) as tc:
        with (
            tc.tile_pool(name="const", bufs=1) as cp,
            tc.tile_pool(name="aug", bufs=1) as ap,
            tc.tile_pool(name="work", bufs=10) as wp,
            tc.tile_pool(name="small", bufs=4) as sp,
            tc.tile_pool(name="ps", bufs=2, space="PSUM") as pp,
        ):
            # ---- input loads (sync/HWDGE queue, batched, in need-order) ----
            # fp8 q/k path first (cheap bytes, unblocks the PE), then the
            # bf16 x for the v-projection, wv, wo.
            wq_sb = cp.tile([128, KIN * 256], f8, tag="wq")
            wq3 = wq_sb[:].rearrange("p (k c) -> p k c", k=KIN)
            wqd3 = wq_d.ap().rearrange("(k p) c -> p k c", k=KIN)
            nc.sync.dma_start(wq3[:, 0:KIN // 2], wqd3[:, 0:KIN // 2])
            xq8_sb = cp.tile([128, KIN * T], f8, tag="xq8")
            xq3 = xq8_sb[:].rearrange("p (k t) -> p k t", k=KIN)
            xqd3 = xq8_d.ap().rearrange("(k p) t -> p k t", k=KIN)
            nc.sync.dma_start(xq3[:, 0:2, 0:W], xqd3[:, 0:2, 0:W])
            nc.sync.dma_start(wq3[:, KIN // 2:KIN], wqd3[:, KIN // 2:KIN])
            for pr in range(1, KIN // 2):
                nc.sync.dma_start(xq3[:, 2 * pr:2 * pr + 2, 0:W],
                                  xqd3[:, 2 * pr:2 * pr + 2, 0:W])
            wk_sb = cp.tile([128, KIN * 256], f8, tag="wk")
            nc.sync.dma_start(
                wk_sb[:].rearrange("p (k c) -> p k c", k=KIN),
                wk_d.ap().rearrange("(k p) c -> p k c", k=KIN))
            for half in range(2):
                k0 = half * (KIN // 2)
                nc.sync.dma_start(xq3[:, k0:k0 + KIN // 2, W:T],
                                  xqd3[:, k0:k0 + KIN // 2, W:T])
            xt_sb = cp.tile([128, KIN * T], bf16, tag="xt")
            xt3 = xt_sb[:].rearrange("p (k t) -> p k t", k=KIN)
            xtd3 = xt_d.ap().rearrange("(k p) t -> p k t", k=KIN)
            for qtr in range(4):
                k0 = qtr * 2
                nc.sync.dma_start(xt3[:, k0:k0 + 2, 0:W], xtd3[:, k0:k0 + 2, 0:W])
            wv_sb = cp.tile([128, KIN * 256], bf16, tag="wv")
            nc.sync.dma_start(
                wv_sb[:].rearrange("p (k c) -> p k c", k=KIN),
                wv_d.ap().rearrange("(k p) c -> p k c", k=KIN))
            for qtr in range(4):
                k0 = qtr * 2
                nc.sync.dma_start(xt3[:, k0:k0 + 2, W:T], xtd3[:, k0:k0 + 2, W:T])
            wo_sb = cp.tile([128, CT * C], bf16, tag="wo")
            nc.sync.dma_start(
                wo_sb[:].rearrange("p (u c) -> p u c", u=CT),
                wo_d.ap().rearrange("(u p) c -> p u c", u=CT))

            def xt_ap(kt, c0, c1):
                return xt_sb[:, kt * T + c0: kt * T + c1]

            # ---- per-head augmented tensors ----
            qaug = [ap.tile([68, T], bf16, tag=f"qaug{h}", name=f"qaug{h}")
                    for h in range(NHL)]
            kaug = [ap.tile([68, T], bf16, tag=f"kaug{h}", name=f"kaug{h}")
                    for h in range(NHL)]
            # v in natural [t, d] layout, one [128, 128] block per (head, kt):
            # cols 0-63 hold v, cols 64-127 stay 1.0 (softmax denominator
            # replicated into psum rows 64-127 by the att@v matmul).
            vaug = ap.tile([128, NHL * KT * 128], bf16, tag="vaug")
            vav = vaug[:].rearrange("p (h k c) -> p h k c", h=NHL, c=128)

            # ---- small constants (Pool SWDGE queue, in need-order) ----
            qb_sb = cp.tile([128, CT], f32, tag="qb")
            nc.gpsimd.dma_start(qb_sb[:], qb_d.ap()[:, :])
            # warm the ACT exp table off the critical path
            warm = sp.tile([128, 2], bf16, tag="warm")
            nc.scalar.activation(warm[:], qb_sb[:, 0:2], Exp)
            nc.gpsimd.memset(vav[:, :, 0:KT // 2, 64:128], 1.0)  # qs0 ones
            tri_sb = cp.tile([128, 128], f32, tag="tri")
            nc.gpsimd.dma_start(tri_sb[:], tri_d.ap()[:, :])
            for h in range(2):
                nc.gpsimd.dma_start(qaug[h][64:68, :], qrow_d.ap()[:, :])
                nc.gpsimd.dma_start(kaug[h][64:68, :],
                                    krows_d.ap()[4 * h:4 * h + 4, :])
            nc.gpsimd.memset(vav[:, :, KT // 2:KT, 64:128], 1.0)  # qs1 ones
            for h in range(2, NHL):
                nc.gpsimd.dma_start(qaug[h][64:68, :], qrow_d.ap()[:, :])
                nc.gpsimd.dma_start(kaug[h][64:68, :],
                                    krows_d.ap()[4 * h:4 * h + 4, :])

            # ---- q/k projection half-chunks ----
            _cp_rot = [0]

            # q/k projections run in fp8 with DoubleRow (2 contraction rows
            # per PE cell -> half the matmul time). Host pre-scales the fp8
            # weights by QKSCALE to stay out of subnormals; the inverse rides
            # the psum->sbuf copy.
            w3q = wq_sb[:].rearrange("p (k c) -> p k c", k=KIN)
            w3k = wk_sb[:].rearrange("p (k c) -> p k c", k=KIN)

            def qkproj_half(which, ct, tsi, half):
                w3, dest = ((w3q, qaug), (w3k, kaug))[which]
                ps_t = pp.tile([128, 512], f32, tag="p", bufs=2,
                               name=f"qkps{which}_{ct}_{tsi}_{half}")
                c0 = tsi * W + half * 512
                for kp in range(KIN // 2):
                    nc.tensor.matmul(
                        ps_t[:],
                        w3[:, 2 * kp:2 * kp + 2, ct * 128:(ct + 1) * 128],
                        xq3[:, 2 * kp:2 * kp + 2, c0:c0 + 512],
                        start=(kp == 0), stop=(kp == KIN // 2 - 1),
                        perf_mode=DR)
                for hl in range(2):
                    h = 2 * ct + hl
                    dst = dest[h][0:64, tsi * W + half * 512:
                                  tsi * W + half * 512 + 512]
                    src = ps_t[hl * 64:(hl + 1) * 64, :]
                    r = _cp_rot[0] = (_cp_rot[0] + 1) % 2
                    if which == 0:
                        bias = qb_sb[hl * 64:(hl + 1) * 64, ct:ct + 1]
                        nc.scalar.activation(dst, src, Ident, bias=bias,
                                             scale=1.0 / QKSCALE)
                    else:
                        if r == 0:
                            nc.scalar.activation(dst, src, Ident,
                                                 scale=1.0 / QKSCALE)
                        else:
                            nc.vector.tensor_scalar_mul(dst, src, 1.0 / QKSCALE)

            def qkproj(which, ct, tsi):
                qkproj_half(which, ct, tsi, 0)
                qkproj_half(which, ct, tsi, 1)

            # ---- v projection tile: natural layout [t, d] into vaug ----
            _sc_rot = [0]

            def vproj_tile(tt):
                ps_t = pp.tile([128, 512], f32, tag="p", bufs=2,
                               name=f"vps{tt}")
                for kt in range(KIN):
                    nc.tensor.matmul(
                        ps_t[:, 0:256],
                        xt_ap(kt, tt * 128, (tt + 1) * 128),
                        wv_sb[:, kt * 256:(kt + 1) * 256],
                        start=(kt == 0), stop=(kt == KIN - 1))
                src = ps_t[:, 0:256].rearrange("p (h c) -> p h c", c=64)
                dst = vaug[:].rearrange("p (h k) -> p h k", k=KT * 128) \
                             [:, :, tt * 128: tt * 128 + 64]
                r = _sc_rot[0] = (_sc_rot[0] + 1) % 2
                if r == 0:
                    nc.scalar.copy(dst, src)
                else:
                    nc.vector.tensor_copy(dst, src)

            # ---- o-projection (partial over this core's 256 channels) ----
            _ost_rot = [0]

            def oproj_mms(tt, half, o_ps):
                c0 = half * 512
                for ct in range(CT):
                    nc.tensor.matmul(
                        o_ps[:],
                        ypair[ct][:, tt * 128:(tt + 1) * 128],
                        wo_sb[:, ct * C + c0: ct * C + c0 + 512],
                        start=(ct == 0), stop=(ct == CT - 1))

            def oproj_half(tt, half, rotate=False):
                o_ps = pp.tile([128, 512], f32, tag="p", bufs=2,
                               name=f"ops{tt}_{half}")
                oproj_mms(tt, half, o_ps)
                c0 = half * 512
                ost = wp.tile([128, 512], bf16, tag="ost",
                              name=f"ost{tt}_{half}")
                r = _ost_rot[0] = (_ost_rot[0] + 1) % 2
                if rotate and r == 1:
                    nc.vector.tensor_copy(ost[:], o_ps[:])
                else:
                    nc.scalar.copy(ost[:], o_ps[:])
                nc.sync.dma_start(
                    out_d.ap()[tt * 128:(tt + 1) * 128, c0:c0 + 512],
                    ost[:])

            def oproj_tailpair(tt0):
                """Two token tiles' o-projection with one batched output DMA;
                copies alternate ACT/DVE (both idle at the tail)."""
                ost2 = wp.tile([128, 2 * C], bf16, tag="ost2", bufs=2,
                               name=f"ost2_{tt0}")
                for u in range(2):
                    for half in range(2):
                        o_ps = pp.tile([128, 512], f32, tag="p", bufs=2,
                                       name=f"ops{tt0 + u}_{half}")
                        oproj_mms(tt0 + u, half, o_ps)
                        dst = ost2[:, u * C + half * 512: u * C + half * 512 + 512]
                        r = _ost_rot[0] = (_ost_rot[0] + 1) % 2
                        if r == 0:
                            nc.scalar.copy(dst, o_ps[:])
                        else:
                            nc.vector.tensor_copy(dst, o_ps[:])
                nc.sync.dma_start(
                    out_d.ap()[tt0 * 128:(tt0 + 2) * 128, :]
                    .rearrange("(u p) c -> p u c", u=2),
                    ost2[:].rearrange("p (u c) -> p u c", u=2))

            ypair = [ap.tile([128, T], bf16, tag=f"ypair{ct}", name=f"ypair{ct}")
                     for ct in range(CT)]

            # ---- attention ----
            def qk_geom(h, qs, kt):
                i0 = qs * W
                off = kt * 128 - i0
                lo = max(0, off)
                hi = min(W, off + 127 + DWIN[h])
                return off, lo, hi

            def attn_steps(h, qs):
                """Generator: emits attention for (h, qs) in packed groups of
                key tiles, yielding at filler-insertion points."""
                i0 = qs * W
                n_kt = (i0 + W) // 128
                kts = [kt for kt in range(n_kt)
                       if qk_geom(h, qs, kt)[2] > qk_geom(h, qs, kt)[1]]
                # pack consecutive key tiles into single-bank score groups
                groups = []
                cur, cw = [], 0
                for kt in kts:
                    off, lo, hi = qk_geom(h, qs, kt)
                    w = hi - lo
                    if cur and cw + w > 512:
                        groups.append(cur)
                        cur, cw = [], 0
                    cur.append((kt, off, lo, hi, cw))
                    cw += w
                groups.append(cur)
                y_half = [pp.tile([128, 512], f32, tag="y", bufs=2,
                                  name=f"y{h}_{qs}_{hf}") for hf in range(2)]
                last_kt_of_half = [None, None]
                for kt in kts:
                    _, lo, hi = qk_geom(h, qs, kt)
                    for (p0, p1) in _bank_pieces(lo, hi):
                        last_kt_of_half[p0 // 512] = kt

                s_tiles = {}

                def emit_qk(gi):
                    s_ps = pp.tile([128, 512], f32, tag="s", bufs=4,
                                   name=f"s{h}_{qs}_{gi}")
                    for (kt, off, lo, hi, base) in groups[gi]:
                        nc.tensor.matmul(
                            s_ps[:, base: base + hi - lo],
                            kaug[h][:, kt * 128: kt * 128 + 128],
                            qaug[h][:, i0 + lo: i0 + hi],
                            start=True, stop=True)
                    s_tiles[gi] = s_ps

                def normalize(hf):
                    y_ps = y_half[hf]
                    recip_sb = sp.tile([64, 512], f32, tag="recip",
                                       name=f"recip{h}_{qs}_{hf}")
                    nc.vector.reciprocal(recip_sb[:], y_ps[64:128, :])
                    ct, hl = h // 2, h % 2
                    nc.vector.tensor_mul(
                        ypair[ct][hl * 64:(hl + 1) * 64,
                                  i0 + hf * 512: i0 + (hf + 1) * 512],
                        y_ps[0:64, :], recip_sb[:])

                emit_qk(0)
                if len(groups) > 1:
                    emit_qk(1)
                hf_started = [False, False]
                for gi, grp in enumerate(groups):
                    if gi + 2 < len(groups):
                        emit_qk(gi + 2)
                    yield
                    s_ps = s_tiles.pop(gi)
                    # diagonal members form a suffix of the group; merge the
                    # uniform-width run into one 3D tensor_add with the tri
                    # mask broadcast (stride-0) along the run dim
                    diag = [(lo, hi, base) for (kt, off, lo, hi, base) in grp
                            if off >= 0]
                    run = [d for d in diag if d[1] - d[0] == diag[0][1] - diag[0][0]] \
                        if diag else []
                    rest = diag[len(run):]
                    if len(run) >= 2:
                        n, w = len(run), run[0][1] - run[0][0]
                        b0 = run[0][2]
                        sview = s_ps[:, b0: b0 + n * w] \
                            .rearrange("p (n c) -> p n c", n=n)[:, :, 0:128]
                        nc.vector.tensor_add(
                            sview, sview,
                            tri_sb[:].unsqueeze(1).broadcast_to([128, n, 128]))
                    elif run:
                        rest = run + rest
                    for (lo, hi, base) in rest:
                        nc.vector.tensor_add(
                            s_ps[:, base: base + 128],
                            s_ps[:, base: base + 128], tri_sb[:])
                    cw = grp[-1][4] + grp[-1][3] - grp[-1][2]
                    pt = wp.tile([128, 512], bf16, tag="pt", bufs=4,
                                 name=f"pt{h}_{qs}_{gi}")
                    nc.scalar.activation(pt[:, 0:cw], s_ps[:, 0:cw], Exp)
                    for (kt, off, lo, hi, base) in grp:
                        for (p0, p1) in _bank_pieces(lo, hi):
                            hf = p0 // 512
                            st = not hf_started[hf]
                            hf_started[hf] = True
                            nc.tensor.matmul(
                                y_half[hf][:, p0 % 512: p0 % 512 + p1 - p0],
                                vaug[:, (h * KT + kt) * 128:
                                     (h * KT + kt) * 128 + 128],
                                pt[:, base + p0 - lo: base + p1 - lo],
                                start=st, stop=False, skip_group_check=True)
                    for hf in range(2):
                        if last_kt_of_half[hf] in [g[0] for g in grp]:
                            normalize(hf)
                    yield

            def run_attn(h, qs, fillers, density=2):
                n = 0
                for _ in attn_steps(h, qs):
                    n += 1
                    if fillers and n % density == 0:
                        fillers.pop(0)()

            # ---- phase schedule ----
            qkproj(0, 0, 0)
            qkproj(1, 0, 0)

            fill = [lambda w=w, h=h: qkproj_half(w, 1, 0, h)
                    for w in (0, 1) for h in (0, 1)]
            fill += [lambda tt=tt: vproj_tile(tt) for tt in range(8)]
            run_attn(0, 0, fill, 2)
            run_attn(1, 0, fill, 2)
            fill += [lambda w=w, h=h: qkproj_half(w, 0, 1, h)
                     for w in (0, 1) for h in (0, 1)]
            run_attn(2, 0, fill, 2)
            fill += [lambda w=w, h=h: qkproj_half(w, 1, 1, h)
                     for w in (0, 1) for h in (0, 1)]
            fill += [lambda tt=tt: vproj_tile(tt) for tt in range(8, TT)]
            run_attn(3, 0, fill, 2)
            for f in fill:
                f()
            ofill = [lambda tt=tt, hf=hf: oproj_half(tt, hf)
                     for tt in range(TT // 2) for hf in (0, 1)]
            run_attn(0, 1, ofill, 2)
            run_attn(1, 1, ofill, 2)
            run_attn(2, 1, ofill, 2)
            run_attn(3, 1, ofill, 2)
            for f in ofill:
                f()
            # batched pairs first; the final tiles go out as fine-grained
            # halves so the last DMA drains in ~1us instead of ~4
            for tt0 in range(TT // 2, TT - 2, 2):
                oproj_tailpair(tt0)
            oproj_half(TT - 2, 0, rotate=True)
            oproj_half(TT - 2, 1, rotate=True)
            # final tile: quarter-copies land on ACT and DVE in parallel so
            # the post-compute drain is one short DMA chain
            for half in range(2):
                o_ps = pp.tile([128, 512], f32, tag="p", bufs=2,
                               name=f"opsF_{half}")
                oproj_mms(TT - 1, half, o_ps)
                ost = wp.tile([128, 512], bf16, tag="ost",
                              name=f"ostF_{half}")
                nc.scalar.copy(ost[:, 0:256], o_ps[:, 0:256])
                nc.vector.tensor_copy(ost[:, 256:512], o_ps[:, 256:512])
                nc.sync.dma_start(
                    out_d.ap()[(TT - 1) * 128:TT * 128,
                               half * 512:half * 512 + 512],
                    ost[:])

    _dedupe_ldweights(nc)
    nc.compile()
    return nc


def _bank_pieces(a, b):
    if a < 512 and b > 512:
        return [(a, 512), (512, b)]
    return [(a, b)]


def _dedupe_ldweights(nc):
    """Remove InstLdweights whose stationary operand is identical to the
    previous PE weight load (nothing in this kernel rewrites a stationary
    tile, so the loaded weights are still valid). Waits/updates of the
    removed load are merged into the next PE instruction."""
    import concourse.mybir as mybir

    PE = mybir.EngineType.PE
    removed = 0
    for blk in nc.m.functions[0].blocks:
        prev_key = None
        pend_waits, pend_updates = [], []
        drop = []
        for inst in blk.instructions:
            if getattr(inst, "engine", None) != PE:
                continue
            tname = type(inst).__name__
            if tname == "InstLdweights":
                key = (str(inst.ins[0]), str(inst.perf_mode),
                       str(inst.tile_position), str(inst.tile_size),
                       str(inst.is_transpose))
                if key == prev_key:
                    si = inst.sync_info
                    if si is not None:
                        pend_waits.extend(list(si.on_wait))
                        pend_updates.extend(list(si.on_update))
                    drop.append(inst)
                else:
                    prev_key = key
            elif tname == "InstMatmult" and not inst.is_transpose:
                if pend_waits or pend_updates:
                    si = inst.sync_info
                    if si is None:
                        inst.sync_info = mybir.SyncInfo(
                            on_wait=pend_waits, on_update=pend_updates)
                    else:
                        si.on_wait = list(si.on_wait) + pend_waits
                        si.on_update = list(si.on_update) + pend_updates
                    pend_waits, pend_updates = [], []
            elif tname == "InstEventSemaphore":
                pass  # transparent to the weight registers
            else:
                prev_key = None  # drain/transpose/branch etc: assume clobber
        assert not (pend_waits or pend_updates), "dangling ldweights syncs"
        for inst in drop:
            blk.instructions.remove(inst)
        removed += len(drop)
    return removed


def _get_nc():
    if "nc" not in _CACHE:
        _CACHE["nc"] = _build_nc()
    return _CACHE["nc"]


def _host_inputs(x, q_w, q_b, kv_w, kv_b, o_w, o_b):
    """Build the 8 per-core input dicts."""
    x = np.asarray(x, np.float32)
    q_w = np.asarray(q_w, np.float32)
    q_b = np.asarray(q_b, np.float32)
    kv_w = np.asarray(kv_w, np.float32)

    F8 = ml_dtypes.float8_e4m3
    xt = [np.ascontiguousarray(x[b].T).astype(BF16) for b in range(B)]
    xq8 = [np.ascontiguousarray(x[b].T).astype(F8) for b in range(B)]
    j = np.arange(T, dtype=np.float32)
    ones = np.ones(T, np.float32)
    qrow = np.stack([-j, ones, ones, ones]).astype(BF16)
    tri = np.where(np.arange(128)[:, None] <= np.arange(128)[None, :],
                   np.float32(0), np.float32(NEG)).astype(np.float32)

    in_maps = []
    for c in range(NCORES):
        b, g = divmod(c, NCORES // B)
        hs = slice(g * 256, (g + 1) * 256)
        slopes = (np.arange(g * 4, g * 4 + 4, dtype=np.float32) + 1.0) / NH
        krows = np.empty((NHL * 4, T), np.float32)
        for hl in range(NHL):
            s = slopes[hl]
            krows[4 * hl + 0] = s
            krows[4 * hl + 1] = s * np.mod(j, 16)
            krows[4 * hl + 2] = s * 16 * np.mod(np.floor(j / 16), 16)
            krows[4 * hl + 3] = s * 256 * np.floor(j / 256)
        in_maps.append({
            "xt": xt[b],
            "xq8": xq8[b],
            "wq": (q_w[:, hs] * np.float32(QKSCALE / np.sqrt(HD))).astype(F8),
            "wk": (kv_w[:, hs] * np.float32(QKSCALE)).astype(F8),
            "wv": kv_w[:, C + g * 256: C + (g + 1) * 256].astype(BF16),
            "wo": np.asarray(o_w, np.float32)[hs, :].astype(BF16),
            "qb": np.ascontiguousarray(
                (q_b[hs] * np.float32(1.0 / np.sqrt(HD))).reshape(CT, 128).T),
            "qrow": qrow,
            "krows": krows.astype(BF16),
            "tri": tri,
        })
    return in_maps


def kernel(x, q_w, q_b, kv_w, kv_b, o_w, o_b):
    from concourse.bass_utils import run_bass_kernel_spmd

    nc = _get_nc()
    in_maps = _host_inputs(x, q_w, q_b, kv_w, kv_b, o_w, o_b)
    res = run_bass_kernel_spmd(nc, in_maps, core_ids=list(range(NCORES)))

    out = np.zeros((B, T, C), np.float32)
    for c in range(NCORES):
        b = c // (NCORES // B)
        out[b] += res.results[c]["o_part"].astype(np.float32)
    # analytic bias terms: v_b flows through softmax (sum=1) into o_w; o_b direct
    const_term = (np.asarray(kv_b, np.float32)[C:] @ np.asarray(o_w, np.float32)
                  + np.asarray(o_b, np.float32))
    out += const_term[None, None, :]
    return out


# revision 33
# speedup vs baseline: 1.0077x; 1.0077x over previous
"""Causal self-attention with ALiBi — Trainium2 Bass kernel, 8-core SPMD.

Problem: y = softmax(mask(q k^T / sqrt(hd) + alibi)) v, with q/kv/o projections.
B=2, T=2048, C=1024, NH=16, HD=64.

Sharding: core c handles batch b = c//4 and heads [4*(c%4), 4*(c%4)+4).
Projections are tensor-parallel over heads; each core emits a partial
o-projection (its 256 channels' contribution); the host sums the 4 partials
per batch (plus the bias terms, which are folded in analytically).

v3 design notes:
- The full ALiBi term rides inside the QK^T matmul via FOUR augmentation row
  pairs: kaug row 64 = slope (pairs with qaug row 64 = -i), and kaug rows
  65..67 = slope*(j%16), slope*16*((j//16)%16), slope*256*(j//256) pairing
  with qaug ones-rows. Each key-side value has an integer numerator <= 240 so
  it is EXACT in bf16; the fp32 psum sum reconstructs slope*j exactly.
  Query-side (-i) rounding cancels per-query in softmax. The Exp activation
  then needs no per-key-tile bias, so one exp covers a GROUP of key tiles
  packed back-to-back in one single-bank [128,512] psum tile.
- ALiBi windows tightened to theta=10 e-foldings (host-checked: ~1e-5 err).
- Score tiles are single-bank with bufs=4: the QK stream runs 2 groups ahead
  of the mask/exp/AV chain so the PE never blocks on ACT/DVE.
- k-projection bias dropped (a key-side bias cancels exactly in softmax).
- All projection psums are single-bank halves (double-buffered 8-bank psum:
  4 score + 2 y + 2 proj).
- Engine placement: exp + half the q/k copies on ACT, normalize + the other
  copies on DVE, tri-mask + v-scatter + memsets on Pool.
- Attention is a generator; projection half-chunks and v/o tiles interleave
  as PE filler between attention groups.
- o-projection rows 0..1023 DMA straight from psum to DRAM in fp32 (no
  engine copy); the tail rows 1024..2047 (engines idle by then) go through
  engine copies to bf16.
"""

import numpy as np
import ml_dtypes

B, T, C = 2, 2048, 1024
NH, HD = 16, 64
NCORES = 8
NHL = 4          # heads per core
W = 1024         # query superchunk width
NQS = T // W     # superchunks
KT = T // 128    # key tiles
CT = 2           # channel tiles for q/k projections (256 channels / 128)
KIN = C // 128   # contraction tiles for projections
TT = T // 128    # token tiles
NEG = -1.0e30
THETA = 8        # ALiBi window e-foldings
DWIN = [(THETA * 16 + hl) // (hl + 1) for hl in range(NHL)]
QKSCALE = 64.0   # fp8 weight pre-scale for the q/k projections

BF16 = ml_dtypes.bfloat16

_CACHE = {}


def _build_nc():
    import concourse.mybir as mybir
    import concourse.tile as tile
    from concourse import bacc

    f32 = mybir.dt.float32
    bf16 = mybir.dt.bfloat16
    f8 = mybir.dt.float8e4
    Exp = mybir.ActivationFunctionType.Exp
    Ident = mybir.ActivationFunctionType.Identity
    DR = mybir.MatmulPerfMode.DoubleRow

    nc = bacc.Bacc("TRN2", target_bir_lowering=False, debug=False,
                   enable_asserts=False, num_devices=NCORES)

    xt_d = nc.dram_tensor("xt", [C, T], bf16, kind="ExternalInput")
    xq8_d = nc.dram_tensor("xq8", [C, T], f8, kind="ExternalInput")
    wq_d = nc.dram_tensor("wq", [C, 256], f8, kind="ExternalInput")
    wk_d = nc.dram_tensor("wk", [C, 256], f8, kind="ExternalInput")
    wv_d = nc.dram_tensor("wv", [C, 256], bf16, kind="ExternalInput")
    wo_d = nc.dram_tensor("wo", [256, C], bf16, kind="ExternalInput")
    qb_d = nc.dram_tensor("qb", [128, CT], f32, kind="ExternalInput")
    qrow_d = nc.dram_tensor("qrow", [4, T], bf16, kind="ExternalInput")
    krows_d = nc.dram_tensor("krows", [NHL * 4, T], bf16, kind="ExternalInput")
    tri_d = nc.dram_tensor("tri", [128, 128], f32, kind="ExternalInput")
    out_d = nc.dram_tensor("o_part", [T, C], bf16, kind="ExternalOutput")

    with tile.TileContext(nc) as tc:
        with (
            tc.tile_pool(name="const", bufs=1) as cp,
            tc.tile_pool(name="aug", bufs=1) as ap,
            tc.tile_pool(name="work", bufs=10) as wp,
            tc.tile_pool(name="small", bufs=4) as sp,
            tc.tile_pool(name="ps", bufs=2, space="PSUM") as pp,
        ):
            # ---- input loads (sync/HWDGE queue, batched, in need-order) ----
            # fp8 q/k path first (cheap bytes, unblocks the PE), then the
            # bf16 x for the v-projection, wv, wo.
            wq_sb = cp.tile([128, KIN * 256], f8, tag="wq")
            wq3 = wq_sb[:].rearrange("p (k c) -> p k c", k=KIN)
            wqd3 = wq_d.ap().rearrange("(k p) c -> p k c", k=KIN)
            nc.sync.dma_start(wq3[:, 0:KIN // 2], wqd3[:, 0:KIN // 2])
            xq8_sb = cp.tile([128, KIN * T], f8, tag="xq8")
            xq3 = xq8_sb[:].rearrange("p (k t) -> p k t", k=KIN)
            xqd3 = xq8_d.ap().rearrange("(k p) t -> p k t", k=KIN)
            nc.sync.dma_start(xq3[:, 0:2, 0:W], xqd3[:, 0:2, 0:W])
            nc.sync.dma_start(wq3[:, KIN // 2:KIN], wqd3[:, KIN // 2:KIN])
            for pr in range(1, KIN // 2):
                nc.sync.dma_start(xq3[:, 2 * pr:2 * pr + 2, 0:W],
                                  xqd3[:, 2 * pr:2 * pr + 2, 0:W])
            wk_sb = cp.tile([128, KIN * 256], f8, tag="wk")
            nc.sync.dma_start(
                wk_sb[:].rearrange("p (k c) -> p k c", k=KIN),
                wk_d.ap().rearrange("(k p) c -> p k c", k=KIN))
            for half in range(2):
                k0 = half * (KIN // 2)
                nc.sync.dma_start(xq3[:, k0:k0 + KIN // 2, W:T],
                                  xqd3[:, k0:k0 + KIN // 2, W:T])
            xt_sb = cp.tile([128, KIN * T], bf16, tag="xt")
            xt3 = xt_sb[:].rearrange("p (k t) -> p k t", k=KIN)
            xtd3 = xt_d.ap().rearrange("(k p) t -> p k t", k=KIN)
            for qtr in range(4):
                k0 = qtr * 2
                nc.sync.dma_start(xt3[:, k0:k0 + 2, 0:W], xtd3[:, k0:k0 + 2, 0:W])
            wv_sb = cp.tile([128, KIN * 256], bf16, tag="wv")
            nc.sync.dma_start(
                wv_sb[:].rearrange("p (k c) -> p k c", k=KIN),
                wv_d.ap().rearrange("(k p) c -> p k c", k=KIN))
            for qtr in range(4):
                k0 = qtr * 2
                nc.sync.dma_start(xt3[:, k0:k0 + 2, W:T], xtd3[:, k0:k0 + 2, W:T])
            wo_sb = cp.tile([128, CT * C], bf16, tag="wo")
            nc.sync.dma_start(
                wo_sb[:].rearrange("p (u c) -> p u c", u=CT),
                wo_d.ap().rearrange("(u p) c -> p u c", u=CT))

            def xt_ap(kt, c0, c1):
                return xt_sb[:, kt * T + c0: kt * T + c1]

            # ---- per-head augmented tensors ----
            qaug = [ap.tile([68, T], bf16, tag=f"qaug{h}", name=f"qaug{h}")
                    for h in range(NHL)]
            kaug = [ap.tile([68, T], bf16, tag=f"kaug{h}", name=f"kaug{h}")
                    for h in range(NHL)]
            # v in natural [t, d] layout, one [128, 128] block per (head, kt):
            # cols 0-63 hold v, cols 64-127 stay 1.0 (softmax denominator
            # replicated into psum rows 64-127 by the att@v matmul).
            vaug = ap.tile([128, NHL * KT * 128], bf16, tag="vaug")
            vav = vaug[:].rearrange("p (h k c) -> p h k c", h=NHL, c=128)

            # ---- small constants (Pool SWDGE queue, in need-order) ----
            qb_sb = cp.tile([128, CT], f32, tag="qb")
            nc.gpsimd.dma_start(qb_sb[:], qb_d.ap()[:, :])
            # warm the ACT exp table off the critical path
            warm = sp.tile([128, 2], bf16, tag="warm")
            nc.scalar.activation(warm[:], qb_sb[:, 0:2], Exp)
            nc.gpsimd.memset(vav[:, :, 0:KT // 2, 64:128], 1.0)  # qs0 ones
            tri_sb = cp.tile([128, 128], f32, tag="tri")
            nc.gpsimd.dma_start(tri_sb[:], tri_d.ap()[:, :])
            for h in range(2):
                nc.gpsimd.dma_start(qaug[h][64:68, :], qrow_d.ap()[:, :])
                nc.gpsimd.dma_start(kaug[h][64:68, :],
                                    krows_d.ap()[4 * h:4 * h + 4, :])
            nc.gpsimd.memset(vav[:, :, KT // 2:KT, 64:128], 1.0)  # qs1 ones
            for h in range(2, NHL):
                nc.gpsimd.dma_start(qaug[h][64:68, :], qrow_d.ap()[:, :])
                nc.gpsimd.dma_start(kaug[h][64:68, :],
                                    krows_d.ap()[4 * h:4 * h + 4, :])

            # ---- q/k projection half-chunks ----
            _cp_rot = [0]

            # q/k projections run in fp8 with DoubleRow (2 contraction rows
            # per PE cell -> half the matmul time). Host pre-scales the fp8
            # weights by QKSCALE to stay out of subnormals; the inverse rides
            # the psum->sbuf copy.
            w3q = wq_sb[:].rearrange("p (k c) -> p k c", k=KIN)
            w3k = wk_sb[:].rearrange("p (k c) -> p k c", k=KIN)

            def qkproj_half(which, ct, tsi, half):
                w3, dest = ((w3q, qaug), (w3k, kaug))[which]
                ps_t = pp.tile([128, 512], f32, tag="p", bufs=2,
                               name=f"qkps{which}_{ct}_{tsi}_{half}")
                c0 = tsi * W + half * 512
                for kp in range(KIN // 2):
                    nc.tensor.matmul(
                        ps_t[:],
                        w3[:, 2 * kp:2 * kp + 2, ct * 128:(ct + 1) * 128],
                        xq3[:, 2 * kp:2 * kp + 2, c0:c0 + 512],
                        start=(kp == 0), stop=(kp == KIN // 2 - 1),
                        perf_mode=DR)
                for hl in range(2):
                    h = 2 * ct + hl
                    dst = dest[h][0:64, tsi * W + half * 512:
                                  tsi * W + half * 512 + 512]
                    src = ps_t[hl * 64:(hl + 1) * 64, :]
                    r = _cp_rot[0] = (_cp_rot[0] + 1) % 2
                    if which == 0:
                        bias = qb_sb[hl * 64:(hl + 1) * 64, ct:ct + 1]
                        if r == 0:
                            nc.scalar.activation(dst, src, Ident, bias=bias,
                                                 scale=1.0 / QKSCALE)
                        else:
                            nc.vector.tensor_scalar(
                                dst, src, 1.0 / QKSCALE, bias,
                                mybir.AluOpType.mult, mybir.AluOpType.add)
                    else:
                        if r == 0:
                            nc.scalar.activation(dst, src, Ident,
                                                 scale=1.0 / QKSCALE)
                        else:
                            nc.vector.tensor_scalar_mul(dst, src, 1.0 / QKSCALE)

            def qkproj(which, ct, tsi):
                qkproj_half(which, ct, tsi, 0)
                qkproj_half(which, ct, tsi, 1)

            # ---- v projection tile: natural layout [t, d] into vaug ----
            _sc_rot = [0]

            def vproj_tile(tt):
                ps_t = pp.tile([128, 512], f32, tag="p", bufs=2,
                               name=f"vps{tt}")
                for kt in range(KIN):
                    nc.tensor.matmul(
                        ps_t[:, 0:256],
                        xt_ap(kt, tt * 128, (tt + 1) * 128),
                        wv_sb[:, kt * 256:(kt + 1) * 256],
                        start=(kt == 0), stop=(kt == KIN - 1))
                src = ps_t[:, 0:256].rearrange("p (h c) -> p h c", c=64)
                dst = vaug[:].rearrange("p (h k) -> p h k", k=KT * 128) \
                             [:, :, tt * 128: tt * 128 + 64]
                r = _sc_rot[0] = (_sc_rot[0] + 1) % 2
                if r == 0:
                    nc.scalar.copy(dst, src)
                else:
                    nc.vector.tensor_copy(dst, src)

            # ---- o-projection (partial over this core's 256 channels) ----
            _ost_rot = [0]

            def oproj_mms(tt, half, o_ps):
                c0 = half * 512
                for ct in range(CT):
                    nc.tensor.matmul(
                        o_ps[:],
                        ypair[ct][:, tt * 128:(tt + 1) * 128],
                        wo_sb[:, ct * C + c0: ct * C + c0 + 512],
                        start=(ct == 0), stop=(ct == CT - 1))

            def oproj_half(tt, half, rotate=False):
                o_ps = pp.tile([128, 512], f32, tag="p", bufs=2,
                               name=f"ops{tt}_{half}")
                oproj_mms(tt, half, o_ps)
                c0 = half * 512
                ost = wp.tile([128, 512], bf16, tag="ost",
                              name=f"ost{tt}_{half}")
                r = _ost_rot[0] = (_ost_rot[0] + 1) % 2
                if rotate and r == 1:
                    nc.vector.tensor_copy(ost[:], o_ps[:])
                else:
                    nc.scalar.copy(ost[:], o_ps[:])
                nc.sync.dma_start(
                    out_d.ap()[tt * 128:(tt + 1) * 128, c0:c0 + 512],
                    ost[:])

            def oproj_tailpair(tt0):
                """Two token tiles' o-projection with one batched output DMA;
                copies alternate ACT/DVE (both idle at the tail)."""
                ost2 = wp.tile([128, 2 * C], bf16, tag="ost2", bufs=2,
                               name=f"ost2_{tt0}")
                for u in range(2):
                    for half in range(2):
                        o_ps = pp.tile([128, 512], f32, tag="p", bufs=2,
                                       name=f"ops{tt0 + u}_{half}")
                        oproj_mms(tt0 + u, half, o_ps)
                        dst = ost2[:, u * C + half * 512: u * C + half * 512 + 512]
                        r = _ost_rot[0] = (_ost_rot[0] + 1) % 2
                        if r == 0:
                            nc.scalar.copy(dst, o_ps[:])
                        else:
                            nc.vector.tensor_copy(dst, o_ps[:])
                nc.sync.dma_start(
                    out_d.ap()[tt0 * 128:(tt0 + 2) * 128, :]
                    .rearrange("(u p) c -> p u c", u=2),
                    ost2[:].rearrange("p (u c) -> p u c", u=2))

            ypair = [ap.tile([128, T], bf16, tag=f"ypair{ct}", name=f"ypair{ct}")
                     for ct in range(CT)]

            # ---- attention ----
            def qk_geom(h, qs, kt):
                i0 = qs * W
                off = kt * 128 - i0
                lo = max(0, off)
                hi = min(W, off + 127 + DWIN[h])
                return off, lo, hi

            def attn_steps(h, qs):
                """Generator: emits attention for (h, qs) in packed groups of
                key tiles, yielding at filler-insertion points."""
                i0 = qs * W
                n_kt = (i0 + W) // 128
                kts = [kt for kt in range(n_kt)
                       if qk_geom(h, qs, kt)[2] > qk_geom(h, qs, kt)[1]]
                # pack consecutive key tiles into single-bank score groups
                groups = []
                cur, cw = [], 0
                for kt in kts:
                    off, lo, hi = qk_geom(h, qs, kt)
                    w = hi - lo
                    if cur and cw + w > 512:
                        groups.append(cur)
                        cur, cw = [], 0
                    cur.append((kt, off, lo, hi, cw))
                    cw += w
                groups.append(cur)
                y_half = [pp.tile([128, 512], f32, tag="y", bufs=2,
                                  name=f"y{h}_{qs}_{hf}") for hf in range(2)]
                last_kt_of_half = [None, None]
                for kt in kts:
                    _, lo, hi = qk_geom(h, qs, kt)
                    for (p0, p1) in _bank_pieces(lo, hi):
                        last_kt_of_half[p0 // 512] = kt

                s_tiles = {}

                def emit_qk(gi):
                    s_ps = pp.tile([128, 512], f32, tag="s", bufs=4,
                                   name=f"s{h}_{qs}_{gi}")
                    for (kt, off, lo, hi, base) in groups[gi]:
                        nc.tensor.matmul(
                            s_ps[:, base: base + hi - lo],
                            kaug[h][:, kt * 128: kt * 128 + 128],
                            qaug[h][:, i0 + lo: i0 + hi],
                            start=True, stop=True)
                    s_tiles[gi] = s_ps

                def normalize(hf):
                    y_ps = y_half[hf]
                    recip_sb = sp.tile([64, 512], f32, tag="recip",
                                       name=f"recip{h}_{qs}_{hf}")
                    nc.vector.reciprocal(recip_sb[:], y_ps[64:128, :])
                    ct, hl = h // 2, h % 2
                    nc.vector.tensor_mul(
                        ypair[ct][hl * 64:(hl + 1) * 64,
                                  i0 + hf * 512: i0 + (hf + 1) * 512],
                        y_ps[0:64, :], recip_sb[:])

                emit_qk(0)
                if len(groups) > 1:
                    emit_qk(1)
                hf_started = [False, False]
                for gi, grp in enumerate(groups):
                    if gi + 2 < len(groups):
                        emit_qk(gi + 2)
                    yield
                    s_ps = s_tiles.pop(gi)
                    # diagonal members form a suffix of the group; merge the
                    # uniform-width run into one 3D tensor_add with the tri
                    # mask broadcast (stride-0) along the run dim
                    diag = [(lo, hi, base) for (kt, off, lo, hi, base) in grp
                            if off >= 0]
                    run = [d for d in diag if d[1] - d[0] == diag[0][1] - diag[0][0]] \
                        if diag else []
                    rest = diag[len(run):]
                    if len(run) >= 2:
                        n, w = len(run), run[0][1] - run[0][0]
                        b0 = run[0][2]
                        sview = s_ps[:, b0: b0 + n * w] \
                            .rearrange("p (n c) -> p n c", n=n)[:, :, 0:128]
                        nc.vector.tensor_add(
                            sview, sview,
                            tri_sb[:].unsqueeze(1).broadcast_to([128, n, 128]))
                    elif run:
                        rest = run + rest
                    for (lo, hi, base) in rest:
                        nc.vector.tensor_add(
                            s_ps[:, base: base + 128],
                            s_ps[:, base: base + 128], tri_sb[:])
                    cw = grp[-1][4] + grp[-1][3] - grp[-1][2]
                    pt = wp.tile([128, 512], bf16, tag="pt", bufs=4,
                                 name=f"pt{h}_{qs}_{gi}")
                    nc.scalar.activation(pt[:, 0:cw], s_ps[:, 0:cw], Exp)
                    for (kt, off, lo, hi, base) in grp:
                        for (p0, p1) in _bank_pieces(lo, hi):
                            hf = p0 // 512
                            st = not hf_started[hf]
                            hf_started[hf] = True
                            nc.tensor.matmul(
                                y_half[hf][:, p0 % 512: p0 % 512 + p1 - p0],
                                vaug[:, (h * KT + kt) * 128:
                                     (h * KT + kt) * 128 + 128],
                                pt[:, base + p0 - lo: base + p1 - lo],
                                start=st, stop=False, skip_group_check=True)
                    for hf in range(2):
                        if last_kt_of_half[hf] in [g[0] for g in grp]:
                            normalize(hf)
                    yield

            def run_attn(h, qs, fillers, density=2):
                n = 0
                for _ in attn_steps(h, qs):
                    n += 1
                    if fillers and n % density == 0:
                        fillers.pop(0)()

            # ---- phase schedule ----
            qkproj(0, 0, 0)
            qkproj(1, 0, 0)

            fill = [lambda w=w, h=h: qkproj_half(w, 1, 0, h)
                    for w in (0, 1) for h in (0, 1)]
            fill += [lambda tt=tt: vproj_tile(tt) for tt in range(8)]
            run_attn(0, 0, fill, 2)
            run_attn(1, 0, fill, 2)
            fill += [lambda w=w, h=h: qkproj_half(w, 0, 1, h)
                     for w in (0, 1) for h in (0, 1)]
            run_attn(2, 0, fill, 2)
            fill += [lambda w=w, h=h: qkproj_half(w, 1, 1, h)
                     for w in (0, 1) for h in (0, 1)]
            fill += [lambda tt=tt: vproj_tile(tt) for tt in range(8, TT)]
            run_attn(3, 0, fill, 2)
            for f in fill:
                f()
            ofill = [lambda tt=tt, hf=hf: oproj_half(tt, hf)
                     for tt in range(TT // 2) for hf in (0, 1)]
            run_attn(0, 1, ofill, 2)
            run_attn(1, 1, ofill, 2)
            run_attn(2, 1, ofill, 2)
            run_attn(3, 1, ofill, 2)
            for f in ofill:
                f()
            # batched pairs first; the final tiles go out as fine-grained
            # halves so the last DMA drains in ~1us instead of ~4
            for tt0 in range(TT // 2, TT - 2, 2):
                oproj_tailpair(tt0)
            oproj_half(TT - 2, 0, rotate=True)
            oproj_half(TT - 2, 1, rotate=True)
            # final tile: quarter-copies land on ACT and DVE in parallel so
            # the post-compute drain is one short DMA chain
            for half in range(2):
                o_ps = pp.tile([128, 512], f32, tag="p", bufs=2,
                               name=f"opsF_{half}")
                oproj_mms(TT - 1, half, o_ps)
                ost = wp.tile([128, 512], bf16, tag="ost",
                              name=f"ostF_{half}")
                nc.scalar.copy(ost[:, 0:256], o_ps[:, 0:256])
                nc.vector.tensor_copy(ost[:, 256:512], o_ps[:, 256:512])
                nc.sync.dma_start(
                    out_d.ap()[(TT - 1) * 128:TT * 128,
                               half * 512:half * 512 + 512],
                    ost[:])

    _dedupe_ldweights(nc)
    nc.compile()
    return nc


def _bank_pieces(a, b):
    if a < 512 and b > 512:
        return [(a, 512), (512, b)]
    return [(a, b)]


def _dedupe_ldweights(nc):
    """Remove InstLdweights whose stationary operand is identical to the
    previous PE weight load (nothing in this kernel rewrites a stationary
    tile, so the loaded weights are still valid). Waits/updates of the
    removed load are merged into the next PE instruction."""
    import concourse.mybir as mybir

    PE = mybir.EngineType.PE
    removed = 0
    for blk in nc.m.functions[0].blocks:
        prev_key = None
        pend_waits, pend_updates = [], []
        drop = []
        for inst in blk.instructions:
            if getattr(inst, "engine", None) != PE:
                continue
            tname = type(inst).__name__
            if tname == "InstLdweights":
                key = (str(inst.ins[0]), str(inst.perf_mode),
                       str(inst.tile_position), str(inst.tile_size),
                       str(inst.is_transpose))
                if key == prev_key:
                    si = inst.sync_info
                    if si is not None:
                        pend_waits.extend(list(si.on_wait))
                        pend_updates.extend(list(si.on_update))
                    drop.append(inst)
                else:
                    prev_key = key
            elif tname == "InstMatmult" and not inst.is_transpose:
                if pend_waits or pend_updates:
                    si = inst.sync_info
                    if si is None:
                        inst.sync_info = mybir.SyncInfo(
                            on_wait=pend_waits, on_update=pend_updates)
                    else:
                        si.on_wait = list(si.on_wait) + pend_waits
                        si.on_update = list(si.on_update) + pend_updates
                    pend_waits, pend_updates = [], []
            elif tname == "InstEventSemaphore":
                pass  # transparent to the weight registers
            else:
                prev_key = None  # drain/transpose/branch etc: assume clobber
        assert not (pend_waits or pend_updates), "dangling ldweights syncs"
        for inst in drop:
            blk.instructions.remove(inst)
        removed += len(drop)
    return removed


def _get_nc():
    if "nc" not in _CACHE:
        _CACHE["nc"] = _build_nc()
    return _CACHE["nc"]


def _host_inputs(x, q_w, q_b, kv_w, kv_b, o_w, o_b):
    """Build the 8 per-core input dicts."""
    x = np.asarray(x, np.float32)
    q_w = np.asarray(q_w, np.float32)
    q_b = np.asarray(q_b, np.float32)
    kv_w = np.asarray(kv_w, np.float32)

    F8 = ml_dtypes.float8_e4m3
    xt = [np.ascontiguousarray(x[b].T).astype(BF16) for b in range(B)]
    xq8 = [np.ascontiguousarray(x[b].T).astype(F8) for b in range(B)]
    j = np.arange(T, dtype=np.float32)
    ones = np.ones(T, np.float32)
    qrow = np.stack([-j, ones, ones, ones]).astype(BF16)
    tri = np.where(np.arange(128)[:, None] <= np.arange(128)[None, :],
                   np.float32(0), np.float32(NEG)).astype(np.float32)

    in_maps = []
    for c in range(NCORES):
        b, g = divmod(c, NCORES // B)
        hs = slice(g * 256, (g + 1) * 256)
        slopes = (np.arange(g * 4, g * 4 + 4, dtype=np.float32) + 1.0) / NH
        krows = np.empty((NHL * 4, T), np.float32)
        for hl in range(NHL):
            s = slopes[hl]
            krows[4 * hl + 0] = s
            krows[4 * hl + 1] = s * np.mod(j, 16)
            krows[4 * hl + 2] = s * 16 * np.mod(np.floor(j / 16), 16)
            krows[4 * hl + 3] = s * 256 * np.floor(j / 256)
        in_maps.append({
            "xt": xt[b],
            "xq8": xq8[b],
            "wq": (q_w[:, hs] * np.float32(QKSCALE / np.sqrt(HD))).astype(F8),
            "wk": (kv_w[:, hs] * np.float32(QKSCALE)).astype(F8),
            "wv": kv_w[:, C + g * 256: C + (g + 1) * 256].astype(BF16),
            "wo": np.asarray(o_w, np.float32)[hs, :].astype(BF16),
            "qb": np.ascontiguousarray(
                (q_b[hs] * np.float32(1.0 / np.sqrt(HD))).reshape(CT, 128).T),
            "qrow": qrow,
            "krows": krows.astype(BF16),
            "tri": tri,
        })
    return in_maps


def kernel(x, q_w, q_b, kv_w, kv_b, o_w, o_b):
    from concourse.bass_utils import run_bass_kernel_spmd

    nc = _get_nc()
    in_maps = _host_inputs(x, q_w, q_b, kv_w, kv_b, o_w, o_b)
    res = run_bass_kernel_spmd(nc, in_maps, core_ids=list(range(NCORES)))

    out = np.zeros((B, T, C), np.float32)
    for c in range(NCORES):
        b = c // (NCORES // B)
        out[b] += res.results[c]["o_part"].astype(np.float32)
    # analytic bias terms: v_b flows through softmax (sum=1) into o_w; o_b direct
    const_term = (np.asarray(kv_b, np.float32)[C:] @ np.asarray(o_w, np.float32)
                  + np.asarray(o_b, np.float32))
    out += const_term[None, None, :]
    return out


# revision 34
# speedup vs baseline: 1.0282x; 1.0204x over previous
"""Causal self-attention with ALiBi — Trainium2 Bass kernel, 8-core SPMD.

Problem: y = softmax(mask(q k^T / sqrt(hd) + alibi)) v, with q/kv/o projections.
B=2, T=2048, C=1024, NH=16, HD=64.

Sharding: core c handles batch b = c//4 and heads [4*(c%4), 4*(c%4)+4).
Projections are tensor-parallel over heads; each core emits a partial
o-projection (its 256 channels' contribution); the host sums the 4 partials
per batch (plus the bias terms, which are folded in analytically).

v3 design notes:
- The full ALiBi term rides inside the QK^T matmul via FOUR augmentation row
  pairs: kaug row 64 = slope (pairs with qaug row 64 = -i), and kaug rows
  65..67 = slope*(j%16), slope*16*((j//16)%16), slope*256*(j//256) pairing
  with qaug ones-rows. Each key-side value has an integer numerator <= 240 so
  it is EXACT in bf16; the fp32 psum sum reconstructs slope*j exactly.
  Query-side (-i) rounding cancels per-query in softmax. The Exp activation
  then needs no per-key-tile bias, so one exp covers a GROUP of key tiles
  packed back-to-back in one single-bank [128,512] psum tile.
- ALiBi windows tightened to theta=10 e-foldings (host-checked: ~1e-5 err).
- Score tiles are single-bank with bufs=4: the QK stream runs 2 groups ahead
  of the mask/exp/AV chain so the PE never blocks on ACT/DVE.
- k-projection bias dropped (a key-side bias cancels exactly in softmax).
- All projection psums are single-bank halves (double-buffered 8-bank psum:
  4 score + 2 y + 2 proj).
- Engine placement: exp + half the q/k copies on ACT, normalize + the other
  copies on DVE, tri-mask + v-scatter + memsets on Pool.
- Attention is a generator; projection half-chunks and v/o tiles interleave
  as PE filler between attention groups.
- o-projection rows 0..1023 DMA straight from psum to DRAM in fp32 (no
  engine copy); the tail rows 1024..2047 (engines idle by then) go through
  engine copies to bf16.
"""

import numpy as np
import ml_dtypes

B, T, C = 2, 2048, 1024
NH, HD = 16, 64
NCORES = 8
NHL = 4          # heads per core
W = 1024         # query superchunk width
NQS = T // W     # superchunks
KT = T // 128    # key tiles
CT = 2           # channel tiles for q/k projections (256 channels / 128)
KIN = C // 128   # contraction tiles for projections
TT = T // 128    # token tiles
NEG = -1.0e30
THETA = 8        # ALiBi window e-foldings
DWIN = [(THETA * 16 + hl) // (hl + 1) for hl in range(NHL)]
QKSCALE = 64.0   # fp8 weight pre-scale for the q/k projections

BF16 = ml_dtypes.bfloat16

_CACHE = {}


def _build_nc():
    import concourse.mybir as mybir
    import concourse.tile as tile
    from concourse import bacc

    f32 = mybir.dt.float32
    bf16 = mybir.dt.bfloat16
    f8 = mybir.dt.float8e4
    Exp = mybir.ActivationFunctionType.Exp
    Ident = mybir.ActivationFunctionType.Identity
    DR = mybir.MatmulPerfMode.DoubleRow

    nc = bacc.Bacc("TRN2", target_bir_lowering=False, debug=False,
                   enable_asserts=False, num_devices=NCORES)

    xt_d = nc.dram_tensor("xt", [C, T], bf16, kind="ExternalInput")
    xq8_d = nc.dram_tensor("xq8", [C, T], f8, kind="ExternalInput")
    wq_d = nc.dram_tensor("wq", [C, 256], f8, kind="ExternalInput")
    wk_d = nc.dram_tensor("wk", [C, 256], f8, kind="ExternalInput")
    wv_d = nc.dram_tensor("wv", [C, 256], bf16, kind="ExternalInput")
    wo_d = nc.dram_tensor("wo", [256, C], bf16, kind="ExternalInput")
    qb_d = nc.dram_tensor("qb", [128, CT], f32, kind="ExternalInput")
    qrow_d = nc.dram_tensor("qrow", [4, T], bf16, kind="ExternalInput")
    krows_d = nc.dram_tensor("krows", [NHL * 4, T], bf16, kind="ExternalInput")
    tri_d = nc.dram_tensor("tri", [128, 128], f32, kind="ExternalInput")
    out_d = nc.dram_tensor("o_part", [T, C], bf16, kind="ExternalOutput")

    with tile.TileContext(nc) as tc:
        with (
            tc.tile_pool(name="const", bufs=1) as cp,
            tc.tile_pool(name="aug", bufs=1) as ap,
            tc.tile_pool(name="work", bufs=10) as wp,
            tc.tile_pool(name="small", bufs=4) as sp,
            tc.tile_pool(name="ps", bufs=2, space="PSUM") as pp,
        ):
            # ---- input loads (sync/HWDGE queue, batched, in need-order) ----
            # fp8 q/k path first (cheap bytes, unblocks the PE), then the
            # bf16 x for the v-projection, wv, wo.
            wq_sb = cp.tile([128, KIN * 256], f8, tag="wq")
            wq3 = wq_sb[:].rearrange("p (k c) -> p k c", k=KIN)
            wqd3 = wq_d.ap().rearrange("(k p) c -> p k c", k=KIN)
            nc.sync.dma_start(wq3[:, 0:KIN // 2], wqd3[:, 0:KIN // 2])
            xq8_sb = cp.tile([128, KIN * T], f8, tag="xq8")
            xq3 = xq8_sb[:].rearrange("p (k t) -> p k t", k=KIN)
            xqd3 = xq8_d.ap().rearrange("(k p) t -> p k t", k=KIN)
            nc.sync.dma_start(xq3[:, 0, 0:W], xqd3[:, 0, 0:W])
            nc.sync.dma_start(xq3[:, 1, 0:W], xqd3[:, 1, 0:W])
            nc.sync.dma_start(wq3[:, KIN // 2:KIN], wqd3[:, KIN // 2:KIN])
            for kt in range(2, KIN):
                nc.sync.dma_start(xq3[:, kt, 0:W], xqd3[:, kt, 0:W])
            wk_sb = cp.tile([128, KIN * 256], f8, tag="wk")
            nc.sync.dma_start(
                wk_sb[:].rearrange("p (k c) -> p k c", k=KIN),
                wk_d.ap().rearrange("(k p) c -> p k c", k=KIN))
            for half in range(2):
                k0 = half * (KIN // 2)
                nc.sync.dma_start(xq3[:, k0:k0 + KIN // 2, W:T],
                                  xqd3[:, k0:k0 + KIN // 2, W:T])
            xt_sb = cp.tile([128, KIN * T], bf16, tag="xt")
            xt3 = xt_sb[:].rearrange("p (k t) -> p k t", k=KIN)
            xtd3 = xt_d.ap().rearrange("(k p) t -> p k t", k=KIN)
            for qtr in range(4):
                k0 = qtr * 2
                nc.sync.dma_start(xt3[:, k0:k0 + 2, 0:W], xtd3[:, k0:k0 + 2, 0:W])
            wv_sb = cp.tile([128, KIN * 256], bf16, tag="wv")
            nc.sync.dma_start(
                wv_sb[:].rearrange("p (k c) -> p k c", k=KIN),
                wv_d.ap().rearrange("(k p) c -> p k c", k=KIN))
            for qtr in range(4):
                k0 = qtr * 2
                nc.sync.dma_start(xt3[:, k0:k0 + 2, W:T], xtd3[:, k0:k0 + 2, W:T])
            wo_sb = cp.tile([128, CT * C], bf16, tag="wo")
            nc.sync.dma_start(
                wo_sb[:].rearrange("p (u c) -> p u c", u=CT),
                wo_d.ap().rearrange("(u p) c -> p u c", u=CT))

            def xt_ap(kt, c0, c1):
                return xt_sb[:, kt * T + c0: kt * T + c1]

            # ---- per-head augmented tensors ----
            qaug = [ap.tile([68, T], bf16, tag=f"qaug{h}", name=f"qaug{h}")
                    for h in range(NHL)]
            kaug = [ap.tile([68, T], bf16, tag=f"kaug{h}", name=f"kaug{h}")
                    for h in range(NHL)]
            # v in natural [t, d] layout, one [128, 128] block per (head, kt):
            # cols 0-63 hold v, cols 64-127 stay 1.0 (softmax denominator
            # replicated into psum rows 64-127 by the att@v matmul).
            vaug = ap.tile([128, NHL * KT * 128], bf16, tag="vaug")
            vav = vaug[:].rearrange("p (h k c) -> p h k c", h=NHL, c=128)

            # ---- small constants (Pool SWDGE queue, in need-order) ----
            qb_sb = cp.tile([128, CT], f32, tag="qb")
            nc.gpsimd.dma_start(qb_sb[:], qb_d.ap()[:, :])
            # warm the ACT exp table off the critical path
            warm = sp.tile([128, 2], bf16, tag="warm")
            nc.scalar.activation(warm[:], qb_sb[:, 0:2], Exp)
            nc.gpsimd.memset(vav[:, :, 0:KT // 2, 64:128], 1.0)  # qs0 ones
            tri_sb = cp.tile([128, 128], f32, tag="tri")
            nc.gpsimd.dma_start(tri_sb[:], tri_d.ap()[:, :])
            for h in range(2):
                nc.gpsimd.dma_start(qaug[h][64:68, :], qrow_d.ap()[:, :])
                nc.gpsimd.dma_start(kaug[h][64:68, :],
                                    krows_d.ap()[4 * h:4 * h + 4, :])
            nc.gpsimd.memset(vav[:, :, KT // 2:KT, 64:128], 1.0)  # qs1 ones
            for h in range(2, NHL):
                nc.gpsimd.dma_start(qaug[h][64:68, :], qrow_d.ap()[:, :])
                nc.gpsimd.dma_start(kaug[h][64:68, :],
                                    krows_d.ap()[4 * h:4 * h + 4, :])

            # ---- q/k projection half-chunks ----
            _cp_rot = [0]

            # q/k projections run in fp8 with DoubleRow (2 contraction rows
            # per PE cell -> half the matmul time). Host pre-scales the fp8
            # weights by QKSCALE to stay out of subnormals; the inverse rides
            # the psum->sbuf copy.
            w3q = wq_sb[:].rearrange("p (k c) -> p k c", k=KIN)
            w3k = wk_sb[:].rearrange("p (k c) -> p k c", k=KIN)

            def qkproj_half(which, ct, tsi, half):
                w3, dest = ((w3q, qaug), (w3k, kaug))[which]
                ps_t = pp.tile([128, 512], f32, tag="p", bufs=2,
                               name=f"qkps{which}_{ct}_{tsi}_{half}")
                c0 = tsi * W + half * 512
                for kp in range(KIN // 2):
                    nc.tensor.matmul(
                        ps_t[:],
                        w3[:, 2 * kp:2 * kp + 2, ct * 128:(ct + 1) * 128],
                        xq3[:, 2 * kp:2 * kp + 2, c0:c0 + 512],
                        start=(kp == 0), stop=(kp == KIN // 2 - 1),
                        perf_mode=DR)
                for hl in range(2):
                    h = 2 * ct + hl
                    dst = dest[h][0:64, tsi * W + half * 512:
                                  tsi * W + half * 512 + 512]
                    src = ps_t[hl * 64:(hl + 1) * 64, :]
                    r = _cp_rot[0] = (_cp_rot[0] + 1) % 2
                    if which == 0:
                        bias = qb_sb[hl * 64:(hl + 1) * 64, ct:ct + 1]
                        if r == 0:
                            nc.scalar.activation(dst, src, Ident, bias=bias,
                                                 scale=1.0 / QKSCALE)
                        else:
                            nc.vector.tensor_scalar(
                                dst, src, 1.0 / QKSCALE, bias,
                                mybir.AluOpType.mult, mybir.AluOpType.add)
                    else:
                        if r == 0:
                            nc.scalar.activation(dst, src, Ident,
                                                 scale=1.0 / QKSCALE)
                        else:
                            nc.vector.tensor_scalar_mul(dst, src, 1.0 / QKSCALE)

            def qkproj(which, ct, tsi):
                qkproj_half(which, ct, tsi, 0)
                qkproj_half(which, ct, tsi, 1)

            # ---- v projection tile: natural layout [t, d] into vaug ----
            _sc_rot = [0]

            def vproj_tile(tt):
                ps_t = pp.tile([128, 512], f32, tag="p", bufs=2,
                               name=f"vps{tt}")
                for kt in range(KIN):
                    nc.tensor.matmul(
                        ps_t[:, 0:256],
                        xt_ap(kt, tt * 128, (tt + 1) * 128),
                        wv_sb[:, kt * 256:(kt + 1) * 256],
                        start=(kt == 0), stop=(kt == KIN - 1))
                src = ps_t[:, 0:256].rearrange("p (h c) -> p h c", c=64)
                dst = vaug[:].rearrange("p (h k) -> p h k", k=KT * 128) \
                             [:, :, tt * 128: tt * 128 + 64]
                r = _sc_rot[0] = (_sc_rot[0] + 1) % 2
                if r == 0:
                    nc.scalar.copy(dst, src)
                else:
                    nc.vector.tensor_copy(dst, src)

            # ---- o-projection (partial over this core's 256 channels) ----
            _ost_rot = [0]

            def oproj_mms(tt, half, o_ps):
                c0 = half * 512
                for ct in range(CT):
                    nc.tensor.matmul(
                        o_ps[:],
                        ypair[ct][:, tt * 128:(tt + 1) * 128],
                        wo_sb[:, ct * C + c0: ct * C + c0 + 512],
                        start=(ct == 0), stop=(ct == CT - 1))

            def oproj_half(tt, half, rotate=False):
                o_ps = pp.tile([128, 512], f32, tag="p", bufs=2,
                               name=f"ops{tt}_{half}")
                oproj_mms(tt, half, o_ps)
                c0 = half * 512
                ost = wp.tile([128, 512], bf16, tag="ost",
                              name=f"ost{tt}_{half}")
                r = _ost_rot[0] = (_ost_rot[0] + 1) % 2
                if rotate and r == 1:
                    nc.vector.tensor_copy(ost[:], o_ps[:])
                else:
                    nc.scalar.copy(ost[:], o_ps[:])
                nc.sync.dma_start(
                    out_d.ap()[tt * 128:(tt + 1) * 128, c0:c0 + 512],
                    ost[:])

            def oproj_tailpair(tt0):
                """Two token tiles' o-projection with one batched output DMA;
                copies alternate ACT/DVE (both idle at the tail)."""
                ost2 = wp.tile([128, 2 * C], bf16, tag="ost2", bufs=2,
                               name=f"ost2_{tt0}")
                for u in range(2):
                    for half in range(2):
                        o_ps = pp.tile([128, 512], f32, tag="p", bufs=2,
                                       name=f"ops{tt0 + u}_{half}")
                        oproj_mms(tt0 + u, half, o_ps)
                        dst = ost2[:, u * C + half * 512: u * C + half * 512 + 512]
                        r = _ost_rot[0] = (_ost_rot[0] + 1) % 2
                        if r == 0:
                            nc.scalar.copy(dst, o_ps[:])
                        else:
                            nc.vector.tensor_copy(dst, o_ps[:])
                nc.sync.dma_start(
                    out_d.ap()[tt0 * 128:(tt0 + 2) * 128, :]
                    .rearrange("(u p) c -> p u c", u=2),
                    ost2[:].rearrange("p (u c) -> p u c", u=2))

            ypair = [ap.tile([128, T], bf16, tag=f"ypair{ct}", name=f"ypair{ct}")
                     for ct in range(CT)]

            # ---- attention ----
            def qk_geom(h, qs, kt):
                i0 = qs * W
                off = kt * 128 - i0
                lo = max(0, off)
                hi = min(W, off + 127 + DWIN[h])
                return off, lo, hi

            def attn_steps(h, qs):
                """Generator: emits attention for (h, qs) in packed groups of
                key tiles, yielding at filler-insertion points."""
                i0 = qs * W
                n_kt = (i0 + W) // 128
                kts = [kt for kt in range(n_kt)
                       if qk_geom(h, qs, kt)[2] > qk_geom(h, qs, kt)[1]]
                # pack consecutive key tiles into single-bank score groups
                groups = []
                cur, cw = [], 0
                for kt in kts:
                    off, lo, hi = qk_geom(h, qs, kt)
                    w = hi - lo
                    if cur and cw + w > 512:
                        groups.append(cur)
                        cur, cw = [], 0
                    cur.append((kt, off, lo, hi, cw))
                    cw += w
                groups.append(cur)
                y_half = [pp.tile([128, 512], f32, tag="y", bufs=2,
                                  name=f"y{h}_{qs}_{hf}") for hf in range(2)]
                last_kt_of_half = [None, None]
                for kt in kts:
                    _, lo, hi = qk_geom(h, qs, kt)
                    for (p0, p1) in _bank_pieces(lo, hi):
                        last_kt_of_half[p0 // 512] = kt

                s_tiles = {}

                def emit_qk(gi):
                    s_ps = pp.tile([128, 512], f32, tag="s", bufs=4,
                                   name=f"s{h}_{qs}_{gi}")
                    for (kt, off, lo, hi, base) in groups[gi]:
                        nc.tensor.matmul(
                            s_ps[:, base: base + hi - lo],
                            kaug[h][:, kt * 128: kt * 128 + 128],
                            qaug[h][:, i0 + lo: i0 + hi],
                            start=True, stop=True)
                    s_tiles[gi] = s_ps

                def normalize(hf):
                    y_ps = y_half[hf]
                    recip_sb = sp.tile([64, 512], f32, tag="recip",
                                       name=f"recip{h}_{qs}_{hf}")
                    nc.vector.reciprocal(recip_sb[:], y_ps[64:128, :])
                    ct, hl = h // 2, h % 2
                    nc.vector.tensor_mul(
                        ypair[ct][hl * 64:(hl + 1) * 64,
                                  i0 + hf * 512: i0 + (hf + 1) * 512],
                        y_ps[0:64, :], recip_sb[:])

                emit_qk(0)
                if len(groups) > 1:
                    emit_qk(1)
                hf_started = [False, False]
                for gi, grp in enumerate(groups):
                    if gi + 2 < len(groups):
                        emit_qk(gi + 2)
                    yield
                    s_ps = s_tiles.pop(gi)
                    # diagonal members form a suffix of the group; merge the
                    # uniform-width run into one 3D tensor_add with the tri
                    # mask broadcast (stride-0) along the run dim
                    diag = [(lo, hi, base) for (kt, off, lo, hi, base) in grp
                            if off >= 0]
                    run = [d for d in diag if d[1] - d[0] == diag[0][1] - diag[0][0]] \
                        if diag else []
                    rest = diag[len(run):]
                    if len(run) >= 2:
                        n, w = len(run), run[0][1] - run[0][0]
                        b0 = run[0][2]
                        sview = s_ps[:, b0: b0 + n * w] \
                            .rearrange("p (n c) -> p n c", n=n)[:, :, 0:128]
                        nc.vector.tensor_add(
                            sview, sview,
                            tri_sb[:].unsqueeze(1).broadcast_to([128, n, 128]))
                    elif run:
                        rest = run + rest
                    for (lo, hi, base) in rest:
                        nc.vector.tensor_add(
                            s_ps[:, base: base + 128],
                            s_ps[:, base: base + 128], tri_sb[:])
                    cw = grp[-1][4] + grp[-1][3] - grp[-1][2]
                    pt = wp.tile([128, 512], bf16, tag="pt", bufs=4,
                                 name=f"pt{h}_{qs}_{gi}")
                    nc.scalar.activation(pt[:, 0:cw], s_ps[:, 0:cw], Exp)
                    for (kt, off, lo, hi, base) in grp:
                        for (p0, p1) in _bank_pieces(lo, hi):
                            hf = p0 // 512
                            st = not hf_started[hf]
                            hf_started[hf] = True
                            nc.tensor.matmul(
                                y_half[hf][:, p0 % 512: p0 % 512 + p1 - p0],
                                vaug[:, (h * KT + kt) * 128:
                                     (h * KT + kt) * 128 + 128],
                                pt[:, base + p0 - lo: base + p1 - lo],
                                start=st, stop=False, skip_group_check=True)
                    for hf in range(2):
                        if last_kt_of_half[hf] in [g[0] for g in grp]:
                            normalize(hf)
                    yield

            def run_attn(h, qs, fillers, density=2):
                n = 0
                for _ in attn_steps(h, qs):
                    n += 1
                    if fillers and n % density == 0:
                        fillers.pop(0)()

            # ---- phase schedule ----
            qkproj(0, 0, 0)
            qkproj(1, 0, 0)

            fill = [lambda w=w, h=h: qkproj_half(w, 1, 0, h)
                    for w in (0, 1) for h in (0, 1)]
            fill += [lambda tt=tt: vproj_tile(tt) for tt in range(8)]
            run_attn(0, 0, fill, 2)
            run_attn(1, 0, fill, 2)
            fill += [lambda w=w, h=h: qkproj_half(w, 0, 1, h)
                     for w in (0, 1) for h in (0, 1)]
            run_attn(2, 0, fill, 2)
            fill += [lambda w=w, h=h: qkproj_half(w, 1, 1, h)
                     for w in (0, 1) for h in (0, 1)]
            fill += [lambda tt=tt: vproj_tile(tt) for tt in range(8, TT)]
            run_attn(3, 0, fill, 2)
            for f in fill:
                f()
            ofill = [lambda tt=tt, hf=hf: oproj_half(tt, hf)
                     for tt in range(TT // 2) for hf in (0, 1)]
            run_attn(0, 1, ofill, 2)
            run_attn(1, 1, ofill, 2)
            run_attn(2, 1, ofill, 2)
            run_attn(3, 1, ofill, 2)
            for f in ofill:
                f()
            # batched pairs first; the final tiles go out as fine-grained
            # halves so the last DMA drains in ~1us instead of ~4
            for tt0 in range(TT // 2, TT - 2, 2):
                oproj_tailpair(tt0)
            oproj_half(TT - 2, 0, rotate=True)
            oproj_half(TT - 2, 1, rotate=True)
            # final tile: quarter-copies land on ACT and DVE in parallel so
            # the post-compute drain is one short DMA chain
            for half in range(2):
                o_ps = pp.tile([128, 512], f32, tag="p", bufs=2,
                               name=f"opsF_{half}")
                oproj_mms(TT - 1, half, o_ps)
                ost = wp.tile([128, 512], bf16, tag="ost",
                              name=f"ostF_{half}")
                nc.scalar.copy(ost[:, 0:256], o_ps[:, 0:256])
                nc.vector.tensor_copy(ost[:, 256:512], o_ps[:, 256:512])
                nc.sync.dma_start(
                    out_d.ap()[(TT - 1) * 128:TT * 128,
                               half * 512:half * 512 + 512],
                    ost[:])

    _dedupe_ldweights(nc)
    nc.compile()
    return nc


def _bank_pieces(a, b):
    if a < 512 and b > 512:
        return [(a, 512), (512, b)]
    return [(a, b)]


def _dedupe_ldweights(nc):
    """Remove InstLdweights whose stationary operand is identical to the
    previous PE weight load (nothing in this kernel rewrites a stationary
    tile, so the loaded weights are still valid). Waits/updates of the
    removed load are merged into the next PE instruction."""
    import concourse.mybir as mybir

    PE = mybir.EngineType.PE
    removed = 0
    for blk in nc.m.functions[0].blocks:
        prev_key = None
        pend_waits, pend_updates = [], []
        drop = []
        for inst in blk.instructions:
            if getattr(inst, "engine", None) != PE:
                continue
            tname = type(inst).__name__
            if tname == "InstLdweights":
                key = (str(inst.ins[0]), str(inst.perf_mode),
                       str(inst.tile_position), str(inst.tile_size),
                       str(inst.is_transpose))
                if key == prev_key:
                    si = inst.sync_info
                    if si is not None:
                        pend_waits.extend(list(si.on_wait))
                        pend_updates.extend(list(si.on_update))
                    drop.append(inst)
                else:
                    prev_key = key
            elif tname == "InstMatmult" and not inst.is_transpose:
                if pend_waits or pend_updates:
                    si = inst.sync_info
                    if si is None:
                        inst.sync_info = mybir.SyncInfo(
                            on_wait=pend_waits, on_update=pend_updates)
                    else:
                        si.on_wait = list(si.on_wait) + pend_waits
                        si.on_update = list(si.on_update) + pend_updates
                    pend_waits, pend_updates = [], []
            elif tname == "InstEventSemaphore":
                pass  # transparent to the weight registers
            else:
                prev_key = None  # drain/transpose/branch etc: assume clobber
        assert not (pend_waits or pend_updates), "dangling ldweights syncs"
        for inst in drop:
            blk.instructions.remove(inst)
        removed += len(drop)
    return removed


def _get_nc():
    if "nc" not in _CACHE:
        _CACHE["nc"] = _build_nc()
    return _CACHE["nc"]


def _host_inputs(x, q_w, q_b, kv_w, kv_b, o_w, o_b):
    """Build the 8 per-core input dicts."""
    x = np.asarray(x, np.float32)
    q_w = np.asarray(q_w, np.float32)
    q_b = np.asarray(q_b, np.float32)
    kv_w = np.asarray(kv_w, np.float32)

    F8 = ml_dtypes.float8_e4m3
    xt = [np.ascontiguousarray(x[b].T).astype(BF16) for b in range(B)]
    xq8 = [np.ascontiguousarray(x[b].T).astype(F8) for b in range(B)]
    j = np.arange(T, dtype=np.float32)
    ones = np.ones(T, np.float32)
    qrow = np.stack([-j, ones, ones, ones]).astype(BF16)
    tri = np.where(np.arange(128)[:, None] <= np.arange(128)[None, :],
                   np.float32(0), np.float32(NEG)).astype(np.float32)

    in_maps = []
    for c in range(NCORES):
        b, g = divmod(c, NCORES // B)
        hs = slice(g * 256, (g + 1) * 256)
        slopes = (np.arange(g * 4, g * 4 + 4, dtype=np.float32) + 1.0) / NH
        krows = np.empty((NHL * 4, T), np.float32)
        for hl in range(NHL):
            s = slopes[hl]
            krows[4 * hl + 0] = s
            krows[4 * hl + 1] = s * np.mod(j, 16)
            krows[4 * hl + 2] = s * 16 * np.mod(np.floor(j / 16), 16)
            krows[4 * hl + 3] = s * 256 * np.floor(j / 256)
        in_maps.append({
            "xt": xt[b],
            "xq8": xq8[b],
            "wq": (q_w[:, hs] * np.float32(QKSCALE / np.sqrt(HD))).astype(F8),
            "wk": (kv_w[:, hs] * np.float32(QKSCALE)).astype(F8),
            "wv": kv_w[:, C + g * 256: C + (g + 1) * 256].astype(BF16),
            "wo": np.asarray(o_w, np.float32)[hs, :].astype(BF16),
            "qb": np.ascontiguousarray(
                (q_b[hs] * np.float32(1.0 / np.sqrt(HD))).reshape(CT, 128).T),
            "qrow": qrow,
            "krows": krows.astype(BF16),
            "tri": tri,
        })
    return in_maps


def kernel(x, q_w, q_b, kv_w, kv_b, o_w, o_b):
    from concourse.bass_utils import run_bass_kernel_spmd

    nc = _get_nc()
    in_maps = _host_inputs(x, q_w, q_b, kv_w, kv_b, o_w, o_b)
    res = run_bass_kernel_spmd(nc, in_maps, core_ids=list(range(NCORES)))

    out = np.zeros((B, T, C), np.float32)
    for c in range(NCORES):
        b = c // (NCORES // B)
        out[b] += res.results[c]["o_part"].astype(np.float32)
    # analytic bias terms: v_b flows through softmax (sum=1) into o_w; o_b direct
    const_term = (np.asarray(kv_b, np.float32)[C:] @ np.asarray(o_w, np.float32)
                  + np.asarray(o_b, np.float32))
    out += const_term[None, None, :]
    return out


# revision 35
# speedup vs baseline: 1.0321x; 1.0038x over previous
"""Causal self-attention with ALiBi — Trainium2 Bass kernel, 8-core SPMD.

Problem: y = softmax(mask(q k^T / sqrt(hd) + alibi)) v, with q/kv/o projections.
B=2, T=2048, C=1024, NH=16, HD=64.

Sharding: core c handles batch b = c//4 and heads [4*(c%4), 4*(c%4)+4).
Projections are tensor-parallel over heads; each core emits a partial
o-projection (its 256 channels' contribution); the host sums the 4 partials
per batch (plus the bias terms, which are folded in analytically).

v3 design notes:
- The full ALiBi term rides inside the QK^T matmul via FOUR augmentation row
  pairs: kaug row 64 = slope (pairs with qaug row 64 = -i), and kaug rows
  65..67 = slope*(j%16), slope*16*((j//16)%16), slope*256*(j//256) pairing
  with qaug ones-rows. Each key-side value has an integer numerator <= 240 so
  it is EXACT in bf16; the fp32 psum sum reconstructs slope*j exactly.
  Query-side (-i) rounding cancels per-query in softmax. The Exp activation
  then needs no per-key-tile bias, so one exp covers a GROUP of key tiles
  packed back-to-back in one single-bank [128,512] psum tile.
- ALiBi windows tightened to theta=10 e-foldings (host-checked: ~1e-5 err).
- Score tiles are single-bank with bufs=4: the QK stream runs 2 groups ahead
  of the mask/exp/AV chain so the PE never blocks on ACT/DVE.
- k-projection bias dropped (a key-side bias cancels exactly in softmax).
- All projection psums are single-bank halves (double-buffered 8-bank psum:
  4 score + 2 y + 2 proj).
- Engine placement: exp + half the q/k copies on ACT, normalize + the other
  copies on DVE, tri-mask + v-scatter + memsets on Pool.
- Attention is a generator; projection half-chunks and v/o tiles interleave
  as PE filler between attention groups.
- o-projection rows 0..1023 DMA straight from psum to DRAM in fp32 (no
  engine copy); the tail rows 1024..2047 (engines idle by then) go through
  engine copies to bf16.
"""

import numpy as np
import ml_dtypes

B, T, C = 2, 2048, 1024
NH, HD = 16, 64
NCORES = 8
NHL = 4          # heads per core
W = 1024         # query superchunk width
NQS = T // W     # superchunks
KT = T // 128    # key tiles
CT = 2           # channel tiles for q/k projections (256 channels / 128)
KIN = C // 128   # contraction tiles for projections
TT = T // 128    # token tiles
NEG = -1.0e30
THETA = 8        # ALiBi window e-foldings
DWIN = [(THETA * 16 + hl) // (hl + 1) for hl in range(NHL)]
QKSCALE = 64.0   # fp8 weight pre-scale for the q/k projections

BF16 = ml_dtypes.bfloat16

_CACHE = {}


def _build_nc():
    import concourse.mybir as mybir
    import concourse.tile as tile
    from concourse import bacc

    f32 = mybir.dt.float32
    bf16 = mybir.dt.bfloat16
    f8 = mybir.dt.float8e4
    Exp = mybir.ActivationFunctionType.Exp
    Ident = mybir.ActivationFunctionType.Identity
    DR = mybir.MatmulPerfMode.DoubleRow

    nc = bacc.Bacc("TRN2", target_bir_lowering=False, debug=False,
                   enable_asserts=False, num_devices=NCORES)

    xt_d = nc.dram_tensor("xt", [C, T], bf16, kind="ExternalInput")
    xq8_d = nc.dram_tensor("xq8", [C, T], f8, kind="ExternalInput")
    wq_d = nc.dram_tensor("wq", [C, 256], f8, kind="ExternalInput")
    wk_d = nc.dram_tensor("wk", [C, 256], f8, kind="ExternalInput")
    wv_d = nc.dram_tensor("wv", [C, 256], bf16, kind="ExternalInput")
    wo_d = nc.dram_tensor("wo", [256, C], bf16, kind="ExternalInput")
    qb_d = nc.dram_tensor("qb", [128, CT], f32, kind="ExternalInput")
    qrow_d = nc.dram_tensor("qrow", [4, T], bf16, kind="ExternalInput")
    krows_d = nc.dram_tensor("krows", [NHL * 4, T], bf16, kind="ExternalInput")
    tri_d = nc.dram_tensor("tri", [128, 128], f32, kind="ExternalInput")
    out_d = nc.dram_tensor("o_part", [T, C], bf16, kind="ExternalOutput")

    with tile.TileContext(nc) as tc:
        with (
            tc.tile_pool(name="const", bufs=1) as cp,
            tc.tile_pool(name="aug", bufs=1) as ap,
            tc.tile_pool(name="work", bufs=10) as wp,
            tc.tile_pool(name="small", bufs=4) as sp,
            tc.tile_pool(name="ps", bufs=2, space="PSUM") as pp,
        ):
            # ---- input loads (sync/HWDGE queue, batched, in need-order) ----
            # fp8 q/k path first (cheap bytes, unblocks the PE), then the
            # bf16 x for the v-projection, wv, wo.
            wq_sb = cp.tile([128, KIN * 256], f8, tag="wq")
            wq3 = wq_sb[:].rearrange("p (k c) -> p k c", k=KIN)
            wqd3 = wq_d.ap().rearrange("(k p) c -> p k c", k=KIN)
            nc.sync.dma_start(wq3[:, 0:KIN // 2], wqd3[:, 0:KIN // 2])
            xq8_sb = cp.tile([128, KIN * T], f8, tag="xq8")
            xq3 = xq8_sb[:].rearrange("p (k t) -> p k t", k=KIN)
            xqd3 = xq8_d.ap().rearrange("(k p) t -> p k t", k=KIN)
            nc.sync.dma_start(xq3[:, 0, 0:W], xqd3[:, 0, 0:W])
            nc.sync.dma_start(xq3[:, 1, 0:W], xqd3[:, 1, 0:W])
            nc.sync.dma_start(wq3[:, KIN // 2:KIN], wqd3[:, KIN // 2:KIN])
            for kt in range(2, KIN):
                nc.sync.dma_start(xq3[:, kt, 0:W], xqd3[:, kt, 0:W])
            wk_sb = cp.tile([128, KIN * 256], f8, tag="wk")
            nc.sync.dma_start(
                wk_sb[:].rearrange("p (k c) -> p k c", k=KIN),
                wk_d.ap().rearrange("(k p) c -> p k c", k=KIN))
            for half in range(2):
                k0 = half * (KIN // 2)
                nc.sync.dma_start(xq3[:, k0:k0 + KIN // 2, W:T],
                                  xqd3[:, k0:k0 + KIN // 2, W:T])
            xt_sb = cp.tile([128, KIN * T], bf16, tag="xt")
            xt3 = xt_sb[:].rearrange("p (k t) -> p k t", k=KIN)
            xtd3 = xt_d.ap().rearrange("(k p) t -> p k t", k=KIN)
            for qtr in range(4):
                k0 = qtr * 2
                nc.sync.dma_start(xt3[:, k0:k0 + 2, 0:W], xtd3[:, k0:k0 + 2, 0:W])
            wv_sb = cp.tile([128, KIN * 256], bf16, tag="wv")
            nc.sync.dma_start(
                wv_sb[:].rearrange("p (k c) -> p k c", k=KIN),
                wv_d.ap().rearrange("(k p) c -> p k c", k=KIN))
            for qtr in range(4):
                k0 = qtr * 2
                nc.sync.dma_start(xt3[:, k0:k0 + 2, W:T], xtd3[:, k0:k0 + 2, W:T])
            wo_sb = cp.tile([128, CT * C], bf16, tag="wo")
            nc.sync.dma_start(
                wo_sb[:].rearrange("p (u c) -> p u c", u=CT),
                wo_d.ap().rearrange("(u p) c -> p u c", u=CT))

            def xt_ap(kt, c0, c1):
                return xt_sb[:, kt * T + c0: kt * T + c1]

            # ---- per-head augmented tensors ----
            qaug = [ap.tile([68, T], bf16, tag=f"qaug{h}", name=f"qaug{h}")
                    for h in range(NHL)]
            kaug = [ap.tile([68, T], bf16, tag=f"kaug{h}", name=f"kaug{h}")
                    for h in range(NHL)]
            # v in natural [t, d] layout, one [128, 128] block per (head, kt):
            # cols 0-63 hold v, cols 64-127 stay 1.0 (softmax denominator
            # replicated into psum rows 64-127 by the att@v matmul).
            vaug = ap.tile([128, NHL * KT * 128], bf16, tag="vaug")
            vav = vaug[:].rearrange("p (h k c) -> p h k c", h=NHL, c=128)

            # ---- small constants (Pool SWDGE queue, in need-order) ----
            qb_sb = cp.tile([128, CT], f32, tag="qb")
            nc.gpsimd.dma_start(qb_sb[:], qb_d.ap()[:, :])
            # warm the ACT exp table off the critical path
            warm = sp.tile([128, 2], bf16, tag="warm")
            nc.scalar.activation(warm[:], qb_sb[:, 0:2], Exp)
            nc.gpsimd.memset(vav[:, :, 0:KT // 2, 64:128], 1.0)  # qs0 ones
            tri_sb = cp.tile([128, 128], f32, tag="tri")
            nc.gpsimd.dma_start(tri_sb[:], tri_d.ap()[:, :])
            for h in range(2):
                nc.gpsimd.dma_start(qaug[h][64:68, :], qrow_d.ap()[:, :])
                nc.gpsimd.dma_start(kaug[h][64:68, :],
                                    krows_d.ap()[4 * h:4 * h + 4, :])
            nc.gpsimd.memset(vav[:, :, KT // 2:KT, 64:128], 1.0)  # qs1 ones
            for h in range(2, NHL):
                nc.gpsimd.dma_start(qaug[h][64:68, :], qrow_d.ap()[:, :])
                nc.gpsimd.dma_start(kaug[h][64:68, :],
                                    krows_d.ap()[4 * h:4 * h + 4, :])

            # ---- q/k projection half-chunks ----
            _cp_rot = [0]

            # q/k projections run in fp8 with DoubleRow (2 contraction rows
            # per PE cell -> half the matmul time). Host pre-scales the fp8
            # weights by QKSCALE to stay out of subnormals; the inverse rides
            # the psum->sbuf copy.
            w3q = wq_sb[:].rearrange("p (k c) -> p k c", k=KIN)
            w3k = wk_sb[:].rearrange("p (k c) -> p k c", k=KIN)

            def qkproj_half(which, ct, tsi, half):
                w3, dest = ((w3q, qaug), (w3k, kaug))[which]
                ps_t = pp.tile([128, 512], f32, tag="p", bufs=2,
                               name=f"qkps{which}_{ct}_{tsi}_{half}")
                c0 = tsi * W + half * 512
                for kp in range(KIN // 2):
                    nc.tensor.matmul(
                        ps_t[:],
                        w3[:, 2 * kp:2 * kp + 2, ct * 128:(ct + 1) * 128],
                        xq3[:, 2 * kp:2 * kp + 2, c0:c0 + 512],
                        start=(kp == 0), stop=(kp == KIN // 2 - 1),
                        perf_mode=DR)
                for hl in range(2):
                    h = 2 * ct + hl
                    dst = dest[h][0:64, tsi * W + half * 512:
                                  tsi * W + half * 512 + 512]
                    src = ps_t[hl * 64:(hl + 1) * 64, :]
                    r = _cp_rot[0] = (_cp_rot[0] + 1) % 2
                    if which == 0:
                        bias = qb_sb[hl * 64:(hl + 1) * 64, ct:ct + 1]
                        if r == 0:
                            nc.scalar.activation(dst, src, Ident, bias=bias,
                                                 scale=1.0 / QKSCALE)
                        else:
                            nc.vector.tensor_scalar(
                                dst, src, 1.0 / QKSCALE, bias,
                                mybir.AluOpType.mult, mybir.AluOpType.add)
                    else:
                        if r == 0:
                            nc.scalar.activation(dst, src, Ident,
                                                 scale=1.0 / QKSCALE)
                        else:
                            nc.vector.tensor_scalar_mul(dst, src, 1.0 / QKSCALE)

            def qkproj(which, ct, tsi):
                qkproj_half(which, ct, tsi, 0)
                qkproj_half(which, ct, tsi, 1)

            # ---- v projection tile: natural layout [t, d] into vaug ----
            _sc_rot = [0]

            def vproj_tile(tt):
                ps_t = pp.tile([128, 512], f32, tag="p", bufs=2,
                               name=f"vps{tt}")
                for kt in range(KIN):
                    nc.tensor.matmul(
                        ps_t[:, 0:256],
                        xt_ap(kt, tt * 128, (tt + 1) * 128),
                        wv_sb[:, kt * 256:(kt + 1) * 256],
                        start=(kt == 0), stop=(kt == KIN - 1))
                src = ps_t[:, 0:256].rearrange("p (h c) -> p h c", c=64)
                dst = vaug[:].rearrange("p (h k) -> p h k", k=KT * 128) \
                             [:, :, tt * 128: tt * 128 + 64]
                r = _sc_rot[0] = (_sc_rot[0] + 1) % 2
                if r == 0:
                    nc.scalar.copy(dst, src)
                else:
                    nc.vector.tensor_copy(dst, src)

            # ---- o-projection (partial over this core's 256 channels) ----
            _ost_rot = [0]

            def oproj_mms(tt, half, o_ps):
                c0 = half * 512
                for ct in range(CT):
                    nc.tensor.matmul(
                        o_ps[:],
                        ypair[ct][:, tt * 128:(tt + 1) * 128],
                        wo_sb[:, ct * C + c0: ct * C + c0 + 512],
                        start=(ct == 0), stop=(ct == CT - 1))

            def oproj_half(tt, half, rotate=False):
                o_ps = pp.tile([128, 512], f32, tag="p", bufs=2,
                               name=f"ops{tt}_{half}")
                oproj_mms(tt, half, o_ps)
                c0 = half * 512
                ost = wp.tile([128, 512], bf16, tag="ost",
                              name=f"ost{tt}_{half}")
                r = _ost_rot[0] = (_ost_rot[0] + 1) % 2
                if rotate and r == 1:
                    nc.vector.tensor_copy(ost[:], o_ps[:])
                else:
                    nc.scalar.copy(ost[:], o_ps[:])
                nc.sync.dma_start(
                    out_d.ap()[tt * 128:(tt + 1) * 128, c0:c0 + 512],
                    ost[:])

            def oproj_tailpair(tt0):
                """Two token tiles' o-projection with one batched output DMA;
                copies alternate ACT/DVE (both idle at the tail)."""
                ost2 = wp.tile([128, 2 * C], bf16, tag="ost2", bufs=2,
                               name=f"ost2_{tt0}")
                for u in range(2):
                    for half in range(2):
                        o_ps = pp.tile([128, 512], f32, tag="p", bufs=2,
                                       name=f"ops{tt0 + u}_{half}")
                        oproj_mms(tt0 + u, half, o_ps)
                        dst = ost2[:, u * C + half * 512: u * C + half * 512 + 512]
                        r = _ost_rot[0] = (_ost_rot[0] + 1) % 2
                        if r == 0:
                            nc.scalar.copy(dst, o_ps[:])
                        else:
                            nc.vector.tensor_copy(dst, o_ps[:])
                nc.sync.dma_start(
                    out_d.ap()[tt0 * 128:(tt0 + 2) * 128, :]
                    .rearrange("(u p) c -> p u c", u=2),
                    ost2[:].rearrange("p (u c) -> p u c", u=2))

            ypair = [ap.tile([128, T], bf16, tag=f"ypair{ct}", name=f"ypair{ct}")
                     for ct in range(CT)]

            # ---- attention ----
            def qk_geom(h, qs, kt):
                i0 = qs * W
                off = kt * 128 - i0
                lo = max(0, off)
                hi = min(W, off + 127 + DWIN[h])
                return off, lo, hi

            def attn_steps(h, qs):
                """Generator: emits attention for (h, qs) in packed groups of
                key tiles, yielding at filler-insertion points."""
                i0 = qs * W
                n_kt = (i0 + W) // 128
                kts = [kt for kt in range(n_kt)
                       if qk_geom(h, qs, kt)[2] > qk_geom(h, qs, kt)[1]]
                # pack consecutive key tiles into single-bank score groups
                groups = []
                cur, cw = [], 0
                for kt in kts:
                    off, lo, hi = qk_geom(h, qs, kt)
                    w = hi - lo
                    if cur and cw + w > 512:
                        groups.append(cur)
                        cur, cw = [], 0
                    cur.append((kt, off, lo, hi, cw))
                    cw += w
                groups.append(cur)
                y_half = [pp.tile([128, 512], f32, tag="y", bufs=2,
                                  name=f"y{h}_{qs}_{hf}") for hf in range(2)]
                last_kt_of_half = [None, None]
                for kt in kts:
                    _, lo, hi = qk_geom(h, qs, kt)
                    for (p0, p1) in _bank_pieces(lo, hi):
                        last_kt_of_half[p0 // 512] = kt

                s_tiles = {}

                def emit_qk(gi):
                    s_ps = pp.tile([128, 512], f32, tag="s", bufs=4,
                                   name=f"s{h}_{qs}_{gi}")
                    for (kt, off, lo, hi, base) in groups[gi]:
                        nc.tensor.matmul(
                            s_ps[:, base: base + hi - lo],
                            kaug[h][:, kt * 128: kt * 128 + 128],
                            qaug[h][:, i0 + lo: i0 + hi],
                            start=True, stop=True)
                    s_tiles[gi] = s_ps

                def normalize(hf):
                    y_ps = y_half[hf]
                    recip_sb = sp.tile([64, 512], f32, tag="recip",
                                       name=f"recip{h}_{qs}_{hf}")
                    nc.vector.reciprocal(recip_sb[:], y_ps[64:128, :])
                    ct, hl = h // 2, h % 2
                    nc.vector.tensor_mul(
                        ypair[ct][hl * 64:(hl + 1) * 64,
                                  i0 + hf * 512: i0 + (hf + 1) * 512],
                        y_ps[0:64, :], recip_sb[:])

                emit_qk(0)
                if len(groups) > 1:
                    emit_qk(1)
                hf_started = [False, False]
                for gi, grp in enumerate(groups):
                    if gi + 2 < len(groups):
                        emit_qk(gi + 2)
                    yield
                    s_ps = s_tiles.pop(gi)
                    # diagonal members form a suffix of the group; merge the
                    # uniform-width run into one 3D tensor_add with the tri
                    # mask broadcast (stride-0) along the run dim
                    diag = [(lo, hi, base) for (kt, off, lo, hi, base) in grp
                            if off >= 0]
                    run = [d for d in diag if d[1] - d[0] == diag[0][1] - diag[0][0]] \
                        if diag else []
                    rest = diag[len(run):]
                    if len(run) >= 2:
                        n, w = len(run), run[0][1] - run[0][0]
                        b0 = run[0][2]
                        sview = s_ps[:, b0: b0 + n * w] \
                            .rearrange("p (n c) -> p n c", n=n)[:, :, 0:128]
                        nc.vector.tensor_add(
                            sview, sview,
                            tri_sb[:].unsqueeze(1).broadcast_to([128, n, 128]))
                    elif run:
                        rest = run + rest
                    for (lo, hi, base) in rest:
                        nc.vector.tensor_add(
                            s_ps[:, base: base + 128],
                            s_ps[:, base: base + 128], tri_sb[:])
                    cw = grp[-1][4] + grp[-1][3] - grp[-1][2]
                    pt = wp.tile([128, 512], bf16, tag="pt", bufs=4,
                                 name=f"pt{h}_{qs}_{gi}")
                    nc.scalar.activation(pt[:, 0:cw], s_ps[:, 0:cw], Exp)
                    for (kt, off, lo, hi, base) in grp:
                        for (p0, p1) in _bank_pieces(lo, hi):
                            hf = p0 // 512
                            st = not hf_started[hf]
                            hf_started[hf] = True
                            nc.tensor.matmul(
                                y_half[hf][:, p0 % 512: p0 % 512 + p1 - p0],
                                vaug[:, (h * KT + kt) * 128:
                                     (h * KT + kt) * 128 + 128],
                                pt[:, base + p0 - lo: base + p1 - lo],
                                start=st, stop=False, skip_group_check=True)
                    for hf in range(2):
                        if last_kt_of_half[hf] in [g[0] for g in grp]:
                            normalize(hf)
                    yield

            def run_attn(h, qs, fillers, density=2):
                n = 0
                for _ in attn_steps(h, qs):
                    n += 1
                    if fillers and n % density == 0:
                        fillers.pop(0)()

            # ---- phase schedule ----
            qkproj(0, 0, 0)
            qkproj(1, 0, 0)

            fill = [lambda w=w, h=h: qkproj_half(w, 1, 0, h)
                    for w in (0, 1) for h in (0, 1)]
            fill += [lambda tt=tt: vproj_tile(tt) for tt in range(8)]
            run_attn(0, 0, fill, 2)
            run_attn(1, 0, fill, 2)
            fill += [lambda w=w, h=h: qkproj_half(w, 0, 1, h)
                     for w in (0, 1) for h in (0, 1)]
            run_attn(2, 0, fill, 2)
            fill += [lambda w=w, h=h: qkproj_half(w, 1, 1, h)
                     for w in (0, 1) for h in (0, 1)]
            fill += [lambda tt=tt: vproj_tile(tt) for tt in range(8, TT)]
            run_attn(3, 0, fill, 2)
            for f in fill:
                f()
            ofill = [lambda tt=tt, hf=hf: oproj_half(tt, hf)
                     for tt in range(TT // 2) for hf in (0, 1)]
            run_attn(0, 1, ofill, 2)
            run_attn(1, 1, ofill, 2)
            run_attn(2, 1, ofill, 2)
            run_attn(3, 1, ofill, 2)
            for f in ofill:
                f()
            # batched pairs first; the final tiles go out as fine-grained
            # halves so the last DMA drains in ~1us instead of ~4
            for tt0 in range(TT // 2, TT - 2, 2):
                oproj_tailpair(tt0)
            for tt in range(TT - 2, TT):
                oproj_half(tt, 0, rotate=True)
                oproj_half(tt, 1, rotate=True)

    _dedupe_ldweights(nc)
    nc.compile()
    return nc


def _bank_pieces(a, b):
    if a < 512 and b > 512:
        return [(a, 512), (512, b)]
    return [(a, b)]


def _dedupe_ldweights(nc):
    """Remove InstLdweights whose stationary operand is identical to the
    previous PE weight load (nothing in this kernel rewrites a stationary
    tile, so the loaded weights are still valid). Waits/updates of the
    removed load are merged into the next PE instruction."""
    import concourse.mybir as mybir

    PE = mybir.EngineType.PE
    removed = 0
    for blk in nc.m.functions[0].blocks:
        prev_key = None
        pend_waits, pend_updates = [], []
        drop = []
        for inst in blk.instructions:
            if getattr(inst, "engine", None) != PE:
                continue
            tname = type(inst).__name__
            if tname == "InstLdweights":
                key = (str(inst.ins[0]), str(inst.perf_mode),
                       str(inst.tile_position), str(inst.tile_size),
                       str(inst.is_transpose))
                if key == prev_key:
                    si = inst.sync_info
                    if si is not None:
                        pend_waits.extend(list(si.on_wait))
                        pend_updates.extend(list(si.on_update))
                    drop.append(inst)
                else:
                    prev_key = key
            elif tname == "InstMatmult" and not inst.is_transpose:
                if pend_waits or pend_updates:
                    si = inst.sync_info
                    if si is None:
                        inst.sync_info = mybir.SyncInfo(
                            on_wait=pend_waits, on_update=pend_updates)
                    else:
                        si.on_wait = list(si.on_wait) + pend_waits
                        si.on_update = list(si.on_update) + pend_updates
                    pend_waits, pend_updates = [], []
            elif tname == "InstEventSemaphore":
                pass  # transparent to the weight registers
            else:
                prev_key = None  # drain/transpose/branch etc: assume clobber
        assert not (pend_waits or pend_updates), "dangling ldweights syncs"
        for inst in drop:
            blk.instructions.remove(inst)
        removed += len(drop)
    return removed


def _get_nc():
    if "nc" not in _CACHE:
        _CACHE["nc"] = _build_nc()
    return _CACHE["nc"]


def _host_inputs(x, q_w, q_b, kv_w, kv_b, o_w, o_b):
    """Build the 8 per-core input dicts."""
    x = np.asarray(x, np.float32)
    q_w = np.asarray(q_w, np.float32)
    q_b = np.asarray(q_b, np.float32)
    kv_w = np.asarray(kv_w, np.float32)

    F8 = ml_dtypes.float8_e4m3
    xt = [np.ascontiguousarray(x[b].T).astype(BF16) for b in range(B)]
    xq8 = [np.ascontiguousarray(x[b].T).astype(F8) for b in range(B)]
    j = np.arange(T, dtype=np.float32)
    ones = np.ones(T, np.float32)
    qrow = np.stack([-j, ones, ones, ones]).astype(BF16)
    tri = np.where(np.arange(128)[:, None] <= np.arange(128)[None, :],
                   np.float32(0), np.float32(NEG)).astype(np.float32)

    in_maps = []
    for c in range(NCORES):
        b, g = divmod(c, NCORES // B)
        hs = slice(g * 256, (g + 1) * 256)
        slopes = (np.arange(g * 4, g * 4 + 4, dtype=np.float32) + 1.0) / NH
        krows = np.empty((NHL * 4, T), np.float32)
        for hl in range(NHL):
            s = slopes[hl]
            krows[4 * hl + 0] = s
            krows[4 * hl + 1] = s * np.mod(j, 16)
            krows[4 * hl + 2] = s * 16 * np.mod(np.floor(j / 16), 16)
            krows[4 * hl + 3] = s * 256 * np.floor(j / 256)
        in_maps.append({
            "xt": xt[b],
            "xq8": xq8[b],
            "wq": (q_w[:, hs] * np.float32(QKSCALE / np.sqrt(HD))).astype(F8),
            "wk": (kv_w[:, hs] * np.float32(QKSCALE)).astype(F8),
            "wv": kv_w[:, C + g * 256: C + (g + 1) * 256].astype(BF16),
            "wo": np.asarray(o_w, np.float32)[hs, :].astype(BF16),
            "qb": np.ascontiguousarray(
                (q_b[hs] * np.float32(1.0 / np.sqrt(HD))).reshape(CT, 128).T),
            "qrow": qrow,
            "krows": krows.astype(BF16),
            "tri": tri,
        })
    return in_maps


def kernel(x, q_w, q_b, kv_w, kv_b, o_w, o_b):
    from concourse.bass_utils import run_bass_kernel_spmd

    nc = _get_nc()
    in_maps = _host_inputs(x, q_w, q_b, kv_w, kv_b, o_w, o_b)
    res = run_bass_kernel_spmd(nc, in_maps, core_ids=list(range(NCORES)))

    out = np.zeros((B, T, C), np.float32)
    for c in range(NCORES):
        b = c // (NCORES // B)
        out[b] += res.results[c]["o_part"].astype(np.float32)
    # analytic bias terms: v_b flows through softmax (sum=1) into o_w; o_b direct
    const_term = (np.asarray(kv_b, np.float32)[C:] @ np.asarray(o_w, np.float32)
                  + np.asarray(o_b, np.float32))
    out += const_term[None, None, :]
    return out


# revision 36
# speedup vs baseline: 1.0364x; 1.0042x over previous
"""Causal self-attention with ALiBi — Trainium2 Bass kernel, 8-core SPMD.

Problem: y = softmax(mask(q k^T / sqrt(hd) + alibi)) v, with q/kv/o projections.
B=2, T=2048, C=1024, NH=16, HD=64.

Sharding: core c handles batch b = c//4 and heads [4*(c%4), 4*(c%4)+4).
Projections are tensor-parallel over heads; each core emits a partial
o-projection (its 256 channels' contribution); the host sums the 4 partials
per batch (plus the bias terms, which are folded in analytically).

v3 design notes:
- The full ALiBi term rides inside the QK^T matmul via FOUR augmentation row
  pairs: kaug row 64 = slope (pairs with qaug row 64 = -i), and kaug rows
  65..67 = slope*(j%16), slope*16*((j//16)%16), slope*256*(j//256) pairing
  with qaug ones-rows. Each key-side value has an integer numerator <= 240 so
  it is EXACT in bf16; the fp32 psum sum reconstructs slope*j exactly.
  Query-side (-i) rounding cancels per-query in softmax. The Exp activation
  then needs no per-key-tile bias, so one exp covers a GROUP of key tiles
  packed back-to-back in one single-bank [128,512] psum tile.
- ALiBi windows tightened to theta=10 e-foldings (host-checked: ~1e-5 err).
- Score tiles are single-bank with bufs=4: the QK stream runs 2 groups ahead
  of the mask/exp/AV chain so the PE never blocks on ACT/DVE.
- k-projection bias dropped (a key-side bias cancels exactly in softmax).
- All projection psums are single-bank halves (double-buffered 8-bank psum:
  4 score + 2 y + 2 proj).
- Engine placement: exp + half the q/k copies on ACT, normalize + the other
  copies on DVE, tri-mask + v-scatter + memsets on Pool.
- Attention is a generator; projection half-chunks and v/o tiles interleave
  as PE filler between attention groups.
- o-projection rows 0..1023 DMA straight from psum to DRAM in fp32 (no
  engine copy); the tail rows 1024..2047 (engines idle by then) go through
  engine copies to bf16.
"""

import numpy as np
import ml_dtypes

B, T, C = 2, 2048, 1024
NH, HD = 16, 64
NCORES = 8
NHL = 4          # heads per core
W = 1024         # query superchunk width
NQS = T // W     # superchunks
KT = T // 128    # key tiles
CT = 2           # channel tiles for q/k projections (256 channels / 128)
KIN = C // 128   # contraction tiles for projections
TT = T // 128    # token tiles
NEG = -1.0e30
THETA = 8        # ALiBi window e-foldings
DWIN = [(THETA * 16 + hl) // (hl + 1) for hl in range(NHL)]
QKSCALE = 64.0   # fp8 weight pre-scale for the q/k projections

BF16 = ml_dtypes.bfloat16

_CACHE = {}


def _build_nc():
    import concourse.mybir as mybir
    import concourse.tile as tile
    from concourse import bacc

    f32 = mybir.dt.float32
    bf16 = mybir.dt.bfloat16
    f8 = mybir.dt.float8e4
    Exp = mybir.ActivationFunctionType.Exp
    Ident = mybir.ActivationFunctionType.Identity
    DR = mybir.MatmulPerfMode.DoubleRow

    nc = bacc.Bacc("TRN2", target_bir_lowering=False, debug=False,
                   enable_asserts=False, num_devices=NCORES)

    xt_d = nc.dram_tensor("xt", [C, T], bf16, kind="ExternalInput")
    xq8_d = nc.dram_tensor("xq8", [C, T], f8, kind="ExternalInput")
    wq_d = nc.dram_tensor("wq", [C, 256], f8, kind="ExternalInput")
    wk_d = nc.dram_tensor("wk", [C, 256], f8, kind="ExternalInput")
    wv_d = nc.dram_tensor("wv", [C, 256], bf16, kind="ExternalInput")
    wo_d = nc.dram_tensor("wo", [256, C], bf16, kind="ExternalInput")
    qb_d = nc.dram_tensor("qb", [128, CT], f32, kind="ExternalInput")
    qrow_d = nc.dram_tensor("qrow", [4, T], bf16, kind="ExternalInput")
    krows_d = nc.dram_tensor("krows", [NHL * 4, T], bf16, kind="ExternalInput")
    tri_d = nc.dram_tensor("tri", [128, 128], f32, kind="ExternalInput")
    out_d = nc.dram_tensor("o_part", [T, C], bf16, kind="ExternalOutput")

    with tile.TileContext(nc) as tc:
        with (
            tc.tile_pool(name="const", bufs=1) as cp,
            tc.tile_pool(name="aug", bufs=1) as ap,
            tc.tile_pool(name="work", bufs=10) as wp,
            tc.tile_pool(name="small", bufs=4) as sp,
            tc.tile_pool(name="ps", bufs=2, space="PSUM") as pp,
        ):
            # ---- input loads (sync/HWDGE queue, batched, in need-order) ----
            # fp8 q/k path first (cheap bytes, unblocks the PE), then the
            # bf16 x for the v-projection, wv, wo.
            wq_sb = cp.tile([128, KIN * 256], f8, tag="wq")
            wq3 = wq_sb[:].rearrange("p (k c) -> p k c", k=KIN)
            wqd3 = wq_d.ap().rearrange("(k p) c -> p k c", k=KIN)
            nc.sync.dma_start(wq3[:, 0:KIN // 2], wqd3[:, 0:KIN // 2])
            xq8_sb = cp.tile([128, KIN * T], f8, tag="xq8")
            xq3 = xq8_sb[:].rearrange("p (k t) -> p k t", k=KIN)
            xqd3 = xq8_d.ap().rearrange("(k p) t -> p k t", k=KIN)
            nc.sync.dma_start(xq3[:, 0, 0:W], xqd3[:, 0, 0:W])
            nc.sync.dma_start(wq3[:, KIN // 2:KIN], wqd3[:, KIN // 2:KIN])
            for kt in range(1, KIN):
                nc.sync.dma_start(xq3[:, kt, 0:W], xqd3[:, kt, 0:W])
            wk_sb = cp.tile([128, KIN * 256], f8, tag="wk")
            nc.sync.dma_start(
                wk_sb[:].rearrange("p (k c) -> p k c", k=KIN),
                wk_d.ap().rearrange("(k p) c -> p k c", k=KIN))
            for half in range(2):
                k0 = half * (KIN // 2)
                nc.sync.dma_start(xq3[:, k0:k0 + KIN // 2, W:T],
                                  xqd3[:, k0:k0 + KIN // 2, W:T])
            xt_sb = cp.tile([128, KIN * T], bf16, tag="xt")
            xt3 = xt_sb[:].rearrange("p (k t) -> p k t", k=KIN)
            xtd3 = xt_d.ap().rearrange("(k p) t -> p k t", k=KIN)
            for qtr in range(4):
                k0 = qtr * 2
                nc.sync.dma_start(xt3[:, k0:k0 + 2, 0:W], xtd3[:, k0:k0 + 2, 0:W])
            wv_sb = cp.tile([128, KIN * 256], bf16, tag="wv")
            nc.sync.dma_start(
                wv_sb[:].rearrange("p (k c) -> p k c", k=KIN),
                wv_d.ap().rearrange("(k p) c -> p k c", k=KIN))
            for qtr in range(4):
                k0 = qtr * 2
                nc.sync.dma_start(xt3[:, k0:k0 + 2, W:T], xtd3[:, k0:k0 + 2, W:T])
            wo_sb = cp.tile([128, CT * C], bf16, tag="wo")
            nc.sync.dma_start(
                wo_sb[:].rearrange("p (u c) -> p u c", u=CT),
                wo_d.ap().rearrange("(u p) c -> p u c", u=CT))

            def xt_ap(kt, c0, c1):
                return xt_sb[:, kt * T + c0: kt * T + c1]

            # ---- per-head augmented tensors ----
            qaug = [ap.tile([68, T], bf16, tag=f"qaug{h}", name=f"qaug{h}")
                    for h in range(NHL)]
            kaug = [ap.tile([68, T], bf16, tag=f"kaug{h}", name=f"kaug{h}")
                    for h in range(NHL)]
            # v in natural [t, d] layout, one [128, 128] block per (head, kt):
            # cols 0-63 hold v, cols 64-127 stay 1.0 (softmax denominator
            # replicated into psum rows 64-127 by the att@v matmul).
            vaug = ap.tile([128, NHL * KT * 128], bf16, tag="vaug")
            vav = vaug[:].rearrange("p (h k c) -> p h k c", h=NHL, c=128)

            # ---- small constants (Pool SWDGE queue, in need-order) ----
            qb_sb = cp.tile([128, CT], f32, tag="qb")
            nc.gpsimd.dma_start(qb_sb[:], qb_d.ap()[:, :])
            # warm the ACT exp table off the critical path
            warm = sp.tile([128, 2], bf16, tag="warm")
            nc.scalar.activation(warm[:], qb_sb[:, 0:2], Exp)
            nc.gpsimd.memset(vav[:, :, 0:KT // 2, 64:128], 1.0)  # qs0 ones
            tri_sb = cp.tile([128, 128], f32, tag="tri")
            nc.gpsimd.dma_start(tri_sb[:], tri_d.ap()[:, :])
            for h in range(2):
                nc.gpsimd.dma_start(qaug[h][64:68, :], qrow_d.ap()[:, :])
                nc.gpsimd.dma_start(kaug[h][64:68, :],
                                    krows_d.ap()[4 * h:4 * h + 4, :])
            nc.gpsimd.memset(vav[:, :, KT // 2:KT, 64:128], 1.0)  # qs1 ones
            for h in range(2, NHL):
                nc.gpsimd.dma_start(qaug[h][64:68, :], qrow_d.ap()[:, :])
                nc.gpsimd.dma_start(kaug[h][64:68, :],
                                    krows_d.ap()[4 * h:4 * h + 4, :])

            # ---- q/k projection half-chunks ----
            _cp_rot = [0]

            # q/k projections run in fp8 with DoubleRow (2 contraction rows
            # per PE cell -> half the matmul time). Host pre-scales the fp8
            # weights by QKSCALE to stay out of subnormals; the inverse rides
            # the psum->sbuf copy.
            w3q = wq_sb[:].rearrange("p (k c) -> p k c", k=KIN)
            w3k = wk_sb[:].rearrange("p (k c) -> p k c", k=KIN)

            def qkproj_half(which, ct, tsi, half):
                w3, dest = ((w3q, qaug), (w3k, kaug))[which]
                ps_t = pp.tile([128, 512], f32, tag="p", bufs=2,
                               name=f"qkps{which}_{ct}_{tsi}_{half}")
                c0 = tsi * W + half * 512
                for kp in range(KIN // 2):
                    nc.tensor.matmul(
                        ps_t[:],
                        w3[:, 2 * kp:2 * kp + 2, ct * 128:(ct + 1) * 128],
                        xq3[:, 2 * kp:2 * kp + 2, c0:c0 + 512],
                        start=(kp == 0), stop=(kp == KIN // 2 - 1),
                        perf_mode=DR)
                for hl in range(2):
                    h = 2 * ct + hl
                    dst = dest[h][0:64, tsi * W + half * 512:
                                  tsi * W + half * 512 + 512]
                    src = ps_t[hl * 64:(hl + 1) * 64, :]
                    r = _cp_rot[0] = (_cp_rot[0] + 1) % 2
                    if which == 0:
                        bias = qb_sb[hl * 64:(hl + 1) * 64, ct:ct + 1]
                        if r == 0:
                            nc.scalar.activation(dst, src, Ident, bias=bias,
                                                 scale=1.0 / QKSCALE)
                        else:
                            nc.vector.tensor_scalar(
                                dst, src, 1.0 / QKSCALE, bias,
                                mybir.AluOpType.mult, mybir.AluOpType.add)
                    else:
                        if r == 0:
                            nc.scalar.activation(dst, src, Ident,
                                                 scale=1.0 / QKSCALE)
                        else:
                            nc.vector.tensor_scalar_mul(dst, src, 1.0 / QKSCALE)

            def qkproj(which, ct, tsi):
                qkproj_half(which, ct, tsi, 0)
                qkproj_half(which, ct, tsi, 1)

            # ---- v projection tile: natural layout [t, d] into vaug ----
            _sc_rot = [0]

            def vproj_tile(tt):
                ps_t = pp.tile([128, 512], f32, tag="p", bufs=2,
                               name=f"vps{tt}")
                for kt in range(KIN):
                    nc.tensor.matmul(
                        ps_t[:, 0:256],
                        xt_ap(kt, tt * 128, (tt + 1) * 128),
                        wv_sb[:, kt * 256:(kt + 1) * 256],
                        start=(kt == 0), stop=(kt == KIN - 1))
                src = ps_t[:, 0:256].rearrange("p (h c) -> p h c", c=64)
                dst = vaug[:].rearrange("p (h k) -> p h k", k=KT * 128) \
                             [:, :, tt * 128: tt * 128 + 64]
                r = _sc_rot[0] = (_sc_rot[0] + 1) % 2
                if r == 0:
                    nc.scalar.copy(dst, src)
                else:
                    nc.vector.tensor_copy(dst, src)

            # ---- o-projection (partial over this core's 256 channels) ----
            _ost_rot = [0]

            def oproj_mms(tt, half, o_ps):
                c0 = half * 512
                for ct in range(CT):
                    nc.tensor.matmul(
                        o_ps[:],
                        ypair[ct][:, tt * 128:(tt + 1) * 128],
                        wo_sb[:, ct * C + c0: ct * C + c0 + 512],
                        start=(ct == 0), stop=(ct == CT - 1))

            def oproj_half(tt, half, rotate=False):
                o_ps = pp.tile([128, 512], f32, tag="p", bufs=2,
                               name=f"ops{tt}_{half}")
                oproj_mms(tt, half, o_ps)
                c0 = half * 512
                ost = wp.tile([128, 512], bf16, tag="ost",
                              name=f"ost{tt}_{half}")
                r = _ost_rot[0] = (_ost_rot[0] + 1) % 2
                if rotate and r == 1:
                    nc.vector.tensor_copy(ost[:], o_ps[:])
                else:
                    nc.scalar.copy(ost[:], o_ps[:])
                nc.sync.dma_start(
                    out_d.ap()[tt * 128:(tt + 1) * 128, c0:c0 + 512],
                    ost[:])

            def oproj_tailpair(tt0):
                """Two token tiles' o-projection with one batched output DMA;
                copies alternate ACT/DVE (both idle at the tail)."""
                ost2 = wp.tile([128, 2 * C], bf16, tag="ost2", bufs=2,
                               name=f"ost2_{tt0}")
                for u in range(2):
                    for half in range(2):
                        o_ps = pp.tile([128, 512], f32, tag="p", bufs=2,
                                       name=f"ops{tt0 + u}_{half}")
                        oproj_mms(tt0 + u, half, o_ps)
                        dst = ost2[:, u * C + half * 512: u * C + half * 512 + 512]
                        r = _ost_rot[0] = (_ost_rot[0] + 1) % 2
                        if r == 0:
                            nc.scalar.copy(dst, o_ps[:])
                        else:
                            nc.vector.tensor_copy(dst, o_ps[:])
                nc.sync.dma_start(
                    out_d.ap()[tt0 * 128:(tt0 + 2) * 128, :]
                    .rearrange("(u p) c -> p u c", u=2),
                    ost2[:].rearrange("p (u c) -> p u c", u=2))

            ypair = [ap.tile([128, T], bf16, tag=f"ypair{ct}", name=f"ypair{ct}")
                     for ct in range(CT)]

            # ---- attention ----
            def qk_geom(h, qs, kt):
                i0 = qs * W
                off = kt * 128 - i0
                lo = max(0, off)
                hi = min(W, off + 127 + DWIN[h])
                return off, lo, hi

            def attn_steps(h, qs):
                """Generator: emits attention for (h, qs) in packed groups of
                key tiles, yielding at filler-insertion points."""
                i0 = qs * W
                n_kt = (i0 + W) // 128
                kts = [kt for kt in range(n_kt)
                       if qk_geom(h, qs, kt)[2] > qk_geom(h, qs, kt)[1]]
                # pack consecutive key tiles into single-bank score groups
                groups = []
                cur, cw = [], 0
                for kt in kts:
                    off, lo, hi = qk_geom(h, qs, kt)
                    w = hi - lo
                    if cur and cw + w > 512:
                        groups.append(cur)
                        cur, cw = [], 0
                    cur.append((kt, off, lo, hi, cw))
                    cw += w
                groups.append(cur)
                y_half = [pp.tile([128, 512], f32, tag="y", bufs=2,
                                  name=f"y{h}_{qs}_{hf}") for hf in range(2)]
                last_kt_of_half = [None, None]
                for kt in kts:
                    _, lo, hi = qk_geom(h, qs, kt)
                    for (p0, p1) in _bank_pieces(lo, hi):
                        last_kt_of_half[p0 // 512] = kt

                s_tiles = {}

                def emit_qk(gi):
                    s_ps = pp.tile([128, 512], f32, tag="s", bufs=4,
                                   name=f"s{h}_{qs}_{gi}")
                    for (kt, off, lo, hi, base) in groups[gi]:
                        nc.tensor.matmul(
                            s_ps[:, base: base + hi - lo],
                            kaug[h][:, kt * 128: kt * 128 + 128],
                            qaug[h][:, i0 + lo: i0 + hi],
                            start=True, stop=True)
                    s_tiles[gi] = s_ps

                def normalize(hf):
                    y_ps = y_half[hf]
                    recip_sb = sp.tile([64, 512], f32, tag="recip",
                                       name=f"recip{h}_{qs}_{hf}")
                    nc.vector.reciprocal(recip_sb[:], y_ps[64:128, :])
                    ct, hl = h // 2, h % 2
                    nc.vector.tensor_mul(
                        ypair[ct][hl * 64:(hl + 1) * 64,
                                  i0 + hf * 512: i0 + (hf + 1) * 512],
                        y_ps[0:64, :], recip_sb[:])

                emit_qk(0)
                if len(groups) > 1:
                    emit_qk(1)
                hf_started = [False, False]
                for gi, grp in enumerate(groups):
                    if gi + 2 < len(groups):
                        emit_qk(gi + 2)
                    yield
                    s_ps = s_tiles.pop(gi)
                    # diagonal members form a suffix of the group; merge the
                    # uniform-width run into one 3D tensor_add with the tri
                    # mask broadcast (stride-0) along the run dim
                    diag = [(lo, hi, base) for (kt, off, lo, hi, base) in grp
                            if off >= 0]
                    run = [d for d in diag if d[1] - d[0] == diag[0][1] - diag[0][0]] \
                        if diag else []
                    rest = diag[len(run):]
                    if len(run) >= 2:
                        n, w = len(run), run[0][1] - run[0][0]
                        b0 = run[0][2]
                        sview = s_ps[:, b0: b0 + n * w] \
                            .rearrange("p (n c) -> p n c", n=n)[:, :, 0:128]
                        nc.vector.tensor_add(
                            sview, sview,
                            tri_sb[:].unsqueeze(1).broadcast_to([128, n, 128]))
                    elif run:
                        rest = run + rest
                    for (lo, hi, base) in rest:
                        nc.vector.tensor_add(
                            s_ps[:, base: base + 128],
                            s_ps[:, base: base + 128], tri_sb[:])
                    cw = grp[-1][4] + grp[-1][3] - grp[-1][2]
                    pt = wp.tile([128, 512], bf16, tag="pt", bufs=4,
                                 name=f"pt{h}_{qs}_{gi}")
                    nc.scalar.activation(pt[:, 0:cw], s_ps[:, 0:cw], Exp)
                    for (kt, off, lo, hi, base) in grp:
                        for (p0, p1) in _bank_pieces(lo, hi):
                            hf = p0 // 512
                            st = not hf_started[hf]
                            hf_started[hf] = True
                            nc.tensor.matmul(
                                y_half[hf][:, p0 % 512: p0 % 512 + p1 - p0],
                                vaug[:, (h * KT + kt) * 128:
                                     (h * KT + kt) * 128 + 128],
                                pt[:, base + p0 - lo: base + p1 - lo],
                                start=st, stop=False, skip_group_check=True)
                    for hf in range(2):
                        if last_kt_of_half[hf] in [g[0] for g in grp]:
                            normalize(hf)
                    yield

            def run_attn(h, qs, fillers, density=2):
                n = 0
                for _ in attn_steps(h, qs):
                    n += 1
                    if fillers and n % density == 0:
                        fillers.pop(0)()

            # ---- phase schedule ----
            qkproj(0, 0, 0)
            qkproj(1, 0, 0)

            fill = [lambda w=w, h=h: qkproj_half(w, 1, 0, h)
                    for w in (0, 1) for h in (0, 1)]
            fill += [lambda tt=tt: vproj_tile(tt) for tt in range(8)]
            run_attn(0, 0, fill, 2)
            run_attn(1, 0, fill, 2)
            fill += [lambda w=w, h=h: qkproj_half(w, 0, 1, h)
                     for w in (0, 1) for h in (0, 1)]
            run_attn(2, 0, fill, 2)
            fill += [lambda w=w, h=h: qkproj_half(w, 1, 1, h)
                     for w in (0, 1) for h in (0, 1)]
            fill += [lambda tt=tt: vproj_tile(tt) for tt in range(8, TT)]
            run_attn(3, 0, fill, 2)
            for f in fill:
                f()
            ofill = [lambda tt=tt, hf=hf: oproj_half(tt, hf)
                     for tt in range(TT // 2) for hf in (0, 1)]
            run_attn(0, 1, ofill, 2)
            run_attn(1, 1, ofill, 2)
            run_attn(2, 1, ofill, 2)
            run_attn(3, 1, ofill, 2)
            for f in ofill:
                f()
            # batched pairs first; the final tiles go out as fine-grained
            # halves so the last DMA drains in ~1us instead of ~4
            for tt0 in range(TT // 2, TT - 2, 2):
                oproj_tailpair(tt0)
            for tt in range(TT - 2, TT):
                oproj_half(tt, 0, rotate=True)
                oproj_half(tt, 1, rotate=True)

    _dedupe_ldweights(nc)
    nc.compile()
    return nc


def _bank_pieces(a, b):
    if a < 512 and b > 512:
        return [(a, 512), (512, b)]
    return [(a, b)]


def _dedupe_ldweights(nc):
    """Remove InstLdweights whose stationary operand is identical to the
    previous PE weight load (nothing in this kernel rewrites a stationary
    tile, so the loaded weights are still valid). Waits/updates of the
    removed load are merged into the next PE instruction."""
    import concourse.mybir as mybir

    PE = mybir.EngineType.PE
    removed = 0
    for blk in nc.m.functions[0].blocks:
        prev_key = None
        pend_waits, pend_updates = [], []
        drop = []
        for inst in blk.instructions:
            if getattr(inst, "engine", None) != PE:
                continue
            tname = type(inst).__name__
            if tname == "InstLdweights":
                key = (str(inst.ins[0]), str(inst.perf_mode),
                       str(inst.tile_position), str(inst.tile_size),
                       str(inst.is_transpose))
                if key == prev_key:
                    si = inst.sync_info
                    if si is not None:
                        pend_waits.extend(list(si.on_wait))
                        pend_updates.extend(list(si.on_update))
                    drop.append(inst)
                else:
                    prev_key = key
            elif tname == "InstMatmult" and not inst.is_transpose:
                if pend_waits or pend_updates:
                    si = inst.sync_info
                    if si is None:
                        inst.sync_info = mybir.SyncInfo(
                            on_wait=pend_waits, on_update=pend_updates)
                    else:
                        si.on_wait = list(si.on_wait) + pend_waits
                        si.on_update = list(si.on_update) + pend_updates
                    pend_waits, pend_updates = [], []
            elif tname == "InstEventSemaphore":
                pass  # transparent to the weight registers
            else:
                prev_key = None  # drain/transpose/branch etc: assume clobber
        assert not (pend_waits or pend_updates), "dangling ldweights syncs"
        for inst in drop:
            blk.instructions.remove(inst)
        removed += len(drop)
    return removed


def _get_nc():
    if "nc" not in _CACHE:
        _CACHE["nc"] = _build_nc()
    return _CACHE["nc"]


def _host_inputs(x, q_w, q_b, kv_w, kv_b, o_w, o_b):
    """Build the 8 per-core input dicts."""
    x = np.asarray(x, np.float32)
    q_w = np.asarray(q_w, np.float32)
    q_b = np.asarray(q_b, np.float32)
    kv_w = np.asarray(kv_w, np.float32)

    F8 = ml_dtypes.float8_e4m3
    xt = [np.ascontiguousarray(x[b].T).astype(BF16) for b in range(B)]
    xq8 = [np.ascontiguousarray(x[b].T).astype(F8) for b in range(B)]
    j = np.arange(T, dtype=np.float32)
    ones = np.ones(T, np.float32)
    qrow = np.stack([-j, ones, ones, ones]).astype(BF16)
    tri = np.where(np.arange(128)[:, None] <= np.arange(128)[None, :],
                   np.float32(0), np.float32(NEG)).astype(np.float32)

    in_maps = []
    for c in range(NCORES):
        b, g = divmod(c, NCORES // B)
        hs = slice(g * 256, (g + 1) * 256)
        slopes = (np.arange(g * 4, g * 4 + 4, dtype=np.float32) + 1.0) / NH
        krows = np.empty((NHL * 4, T), np.float32)
        for hl in range(NHL):
            s = slopes[hl]
            krows[4 * hl + 0] = s
            krows[4 * hl + 1] = s * np.mod(j, 16)
            krows[4 * hl + 2] = s * 16 * np.mod(np.floor(j / 16), 16)
            krows[4 * hl + 3] = s * 256 * np.floor(j / 256)
        in_maps.append({
            "xt": xt[b],
            "xq8": xq8[b],
            "wq": (q_w[:, hs] * np.float32(QKSCALE / np.sqrt(HD))).astype(F8),
            "wk": (kv_w[:, hs] * np.float32(QKSCALE)).astype(F8),
            "wv": kv_w[:, C + g * 256: C + (g + 1) * 256].astype(BF16),
            "wo": np.asarray(o_w, np.float32)[hs, :].astype(BF16),
            "qb": np.ascontiguousarray(
                (q_b[hs] * np.float32(1.0 / np.sqrt(HD))).reshape(CT, 128).T),
            "qrow": qrow,
            "krows": krows.astype(BF16),
            "tri": tri,
        })
    return in_maps


def kernel(x, q_w, q_b, kv_w, kv_b, o_w, o_b):
    from concourse.bass_utils import run_bass_kernel_spmd

    nc = _get_nc()
    in_maps = _host_inputs(x, q_w, q_b, kv_w, kv_b, o_w, o_b)
    res = run_bass_kernel_spmd(nc, in_maps, core_ids=list(range(NCORES)))

    out = np.zeros((B, T, C), np.float32)
    for c in range(NCORES):
        b = c // (NCORES // B)
        out[b] += res.results[c]["o_part"].astype(np.float32)
    # analytic bias terms: v_b flows through softmax (sum=1) into o_w; o_b direct
    const_term = (np.asarray(kv_b, np.float32)[C:] @ np.asarray(o_w, np.float32)
                  + np.asarray(o_b, np.float32))
    out += const_term[None, None, :]
    return out


# revision 37
# speedup vs baseline: 1.0400x; 1.0035x over previous
"""Causal self-attention with ALiBi — Trainium2 Bass kernel, 8-core SPMD.

Problem: y = softmax(mask(q k^T / sqrt(hd) + alibi)) v, with q/kv/o projections.
B=2, T=2048, C=1024, NH=16, HD=64.

Sharding: core c handles batch b = c//4 and heads [4*(c%4), 4*(c%4)+4).
Projections are tensor-parallel over heads; each core emits a partial
o-projection (its 256 channels' contribution); the host sums the 4 partials
per batch (plus the bias terms, which are folded in analytically).

v3 design notes:
- The full ALiBi term rides inside the QK^T matmul via FOUR augmentation row
  pairs: kaug row 64 = slope (pairs with qaug row 64 = -i), and kaug rows
  65..67 = slope*(j%16), slope*16*((j//16)%16), slope*256*(j//256) pairing
  with qaug ones-rows. Each key-side value has an integer numerator <= 240 so
  it is EXACT in bf16; the fp32 psum sum reconstructs slope*j exactly.
  Query-side (-i) rounding cancels per-query in softmax. The Exp activation
  then needs no per-key-tile bias, so one exp covers a GROUP of key tiles
  packed back-to-back in one single-bank [128,512] psum tile.
- ALiBi windows tightened to theta=10 e-foldings (host-checked: ~1e-5 err).
- Score tiles are single-bank with bufs=4: the QK stream runs 2 groups ahead
  of the mask/exp/AV chain so the PE never blocks on ACT/DVE.
- k-projection bias dropped (a key-side bias cancels exactly in softmax).
- All projection psums are single-bank halves (double-buffered 8-bank psum:
  4 score + 2 y + 2 proj).
- Engine placement: exp + half the q/k copies on ACT, normalize + the other
  copies on DVE, tri-mask + v-scatter + memsets on Pool.
- Attention is a generator; projection half-chunks and v/o tiles interleave
  as PE filler between attention groups.
- o-projection rows 0..1023 DMA straight from psum to DRAM in fp32 (no
  engine copy); the tail rows 1024..2047 (engines idle by then) go through
  engine copies to bf16.
"""

import numpy as np
import ml_dtypes

B, T, C = 2, 2048, 1024
NH, HD = 16, 64
NCORES = 8
NHL = 4          # heads per core
W = 1024         # query superchunk width
NQS = T // W     # superchunks
KT = T // 128    # key tiles
CT = 2           # channel tiles for q/k projections (256 channels / 128)
KIN = C // 128   # contraction tiles for projections
TT = T // 128    # token tiles
NEG = -1.0e30
THETA = 8        # ALiBi window e-foldings
DWIN = [(THETA * 16 + hl) // (hl + 1) for hl in range(NHL)]
QKSCALE = 64.0   # fp8 weight pre-scale for the q/k projections

BF16 = ml_dtypes.bfloat16

_CACHE = {}


def _build_nc():
    import concourse.mybir as mybir
    import concourse.tile as tile
    from concourse import bacc

    f32 = mybir.dt.float32
    bf16 = mybir.dt.bfloat16
    f8 = mybir.dt.float8e4
    Exp = mybir.ActivationFunctionType.Exp
    Ident = mybir.ActivationFunctionType.Identity
    DR = mybir.MatmulPerfMode.DoubleRow

    nc = bacc.Bacc("TRN2", target_bir_lowering=False, debug=False,
                   enable_asserts=False, num_devices=NCORES)

    xt_d = nc.dram_tensor("xt", [C, T], bf16, kind="ExternalInput")
    xq8_d = nc.dram_tensor("xq8", [C, T], f8, kind="ExternalInput")
    wq_d = nc.dram_tensor("wq", [C, 256], f8, kind="ExternalInput")
    wk_d = nc.dram_tensor("wk", [C, 256], f8, kind="ExternalInput")
    wv_d = nc.dram_tensor("wv", [C, 256], bf16, kind="ExternalInput")
    wo_d = nc.dram_tensor("wo", [256, C], bf16, kind="ExternalInput")
    qb_d = nc.dram_tensor("qb", [128, CT], f32, kind="ExternalInput")
    qrow_d = nc.dram_tensor("qrow", [4, T], bf16, kind="ExternalInput")
    krows_d = nc.dram_tensor("krows", [NHL * 4, T], bf16, kind="ExternalInput")
    tri_d = nc.dram_tensor("tri", [128, 128], f32, kind="ExternalInput")
    out_d = nc.dram_tensor("o_part", [T, C], bf16, kind="ExternalOutput")

    with tile.TileContext(nc) as tc:
        with (
            tc.tile_pool(name="const", bufs=1) as cp,
            tc.tile_pool(name="aug", bufs=1) as ap,
            tc.tile_pool(name="work", bufs=10) as wp,
            tc.tile_pool(name="small", bufs=4) as sp,
            tc.tile_pool(name="ps", bufs=2, space="PSUM") as pp,
        ):
            # ---- input loads (sync/HWDGE queue, batched, in need-order) ----
            # fp8 q/k path first (cheap bytes, unblocks the PE), then the
            # bf16 x for the v-projection, wv, wo.
            wq_sb = cp.tile([128, KIN * 256], f8, tag="wq")
            wq3 = wq_sb[:].rearrange("p (k c) -> p k c", k=KIN)
            wqd3 = wq_d.ap().rearrange("(k p) c -> p k c", k=KIN)
            xq8_sb = cp.tile([128, KIN * T], f8, tag="xq8")
            xq3 = xq8_sb[:].rearrange("p (k t) -> p k t", k=KIN)
            xqd3 = xq8_d.ap().rearrange("(k p) t -> p k t", k=KIN)
            wk_sb = cp.tile([128, KIN * 256], f8, tag="wk")
            wk3 = wk_sb[:].rearrange("p (k c) -> p k c", k=KIN)
            wkd3 = wk_d.ap().rearrange("(k p) c -> p k c", k=KIN)
            # pair-granular xq8 loads (DoubleRow consumes kt pairs)
            # interleaved with the weight halves in first-use order
            nc.sync.dma_start(wq3[:, 0:KIN // 2], wqd3[:, 0:KIN // 2])
            nc.sync.dma_start(xq3[:, 0:2, 0:W], xqd3[:, 0:2, 0:W])
            nc.sync.dma_start(wq3[:, KIN // 2:KIN], wqd3[:, KIN // 2:KIN])
            nc.sync.dma_start(xq3[:, 2:4, 0:W], xqd3[:, 2:4, 0:W])
            nc.sync.dma_start(wk3[:, 0:KIN // 2], wkd3[:, 0:KIN // 2])
            nc.sync.dma_start(xq3[:, 4:6, 0:W], xqd3[:, 4:6, 0:W])
            nc.sync.dma_start(xq3[:, 6:8, 0:W], xqd3[:, 6:8, 0:W])
            nc.sync.dma_start(wk3[:, KIN // 2:KIN], wkd3[:, KIN // 2:KIN])
            for half in range(2):
                k0 = half * (KIN // 2)
                nc.sync.dma_start(xq3[:, k0:k0 + KIN // 2, W:T],
                                  xqd3[:, k0:k0 + KIN // 2, W:T])
            xt_sb = cp.tile([128, KIN * T], bf16, tag="xt")
            xt3 = xt_sb[:].rearrange("p (k t) -> p k t", k=KIN)
            xtd3 = xt_d.ap().rearrange("(k p) t -> p k t", k=KIN)
            for qtr in range(4):
                k0 = qtr * 2
                nc.sync.dma_start(xt3[:, k0:k0 + 2, 0:W], xtd3[:, k0:k0 + 2, 0:W])
            wv_sb = cp.tile([128, KIN * 256], bf16, tag="wv")
            nc.sync.dma_start(
                wv_sb[:].rearrange("p (k c) -> p k c", k=KIN),
                wv_d.ap().rearrange("(k p) c -> p k c", k=KIN))
            for qtr in range(4):
                k0 = qtr * 2
                nc.sync.dma_start(xt3[:, k0:k0 + 2, W:T], xtd3[:, k0:k0 + 2, W:T])
            wo_sb = cp.tile([128, CT * C], bf16, tag="wo")
            nc.sync.dma_start(
                wo_sb[:].rearrange("p (u c) -> p u c", u=CT),
                wo_d.ap().rearrange("(u p) c -> p u c", u=CT))

            def xt_ap(kt, c0, c1):
                return xt_sb[:, kt * T + c0: kt * T + c1]

            # ---- per-head augmented tensors ----
            qaug = [ap.tile([68, T], bf16, tag=f"qaug{h}", name=f"qaug{h}")
                    for h in range(NHL)]
            kaug = [ap.tile([68, T], bf16, tag=f"kaug{h}", name=f"kaug{h}")
                    for h in range(NHL)]
            # v in natural [t, d] layout, one [128, 128] block per (head, kt):
            # cols 0-63 hold v, cols 64-127 stay 1.0 (softmax denominator
            # replicated into psum rows 64-127 by the att@v matmul).
            vaug = ap.tile([128, NHL * KT * 128], bf16, tag="vaug")
            vav = vaug[:].rearrange("p (h k c) -> p h k c", h=NHL, c=128)

            # ---- small constants (Pool SWDGE queue, in need-order) ----
            qb_sb = cp.tile([128, CT], f32, tag="qb")
            nc.gpsimd.dma_start(qb_sb[:], qb_d.ap()[:, :])
            # warm the ACT exp table off the critical path
            warm = sp.tile([128, 2], bf16, tag="warm")
            nc.scalar.activation(warm[:], qb_sb[:, 0:2], Exp)
            tri_sb = cp.tile([128, 128], f32, tag="tri")
            nc.gpsimd.dma_start(tri_sb[:], tri_d.ap()[:, :])
            for h in range(2):
                nc.gpsimd.dma_start(qaug[h][64:68, :], qrow_d.ap()[:, :])
                nc.gpsimd.dma_start(kaug[h][64:68, :],
                                    krows_d.ap()[4 * h:4 * h + 4, :])
            # ones columns interleaved in AV-consumption order
            for qq in range(4):
                nc.gpsimd.memset(
                    vav[:, :, qq * (KT // 4):(qq + 1) * (KT // 4), 64:128], 1.0)
                if qq < 2:
                    h = 2 + qq
                    nc.gpsimd.dma_start(qaug[h][64:68, :], qrow_d.ap()[:, :])
                    nc.gpsimd.dma_start(kaug[h][64:68, :],
                                        krows_d.ap()[4 * h:4 * h + 4, :])

            # ---- q/k projection half-chunks ----
            _cp_rot = [0]

            # q/k projections run in fp8 with DoubleRow (2 contraction rows
            # per PE cell -> half the matmul time). Host pre-scales the fp8
            # weights by QKSCALE to stay out of subnormals; the inverse rides
            # the psum->sbuf copy.
            w3q = wq_sb[:].rearrange("p (k c) -> p k c", k=KIN)
            w3k = wk_sb[:].rearrange("p (k c) -> p k c", k=KIN)

            def qkproj_half(which, ct, tsi, half):
                w3, dest = ((w3q, qaug), (w3k, kaug))[which]
                ps_t = pp.tile([128, 512], f32, tag="p", bufs=2,
                               name=f"qkps{which}_{ct}_{tsi}_{half}")
                c0 = tsi * W + half * 512
                for kp in range(KIN // 2):
                    nc.tensor.matmul(
                        ps_t[:],
                        w3[:, 2 * kp:2 * kp + 2, ct * 128:(ct + 1) * 128],
                        xq3[:, 2 * kp:2 * kp + 2, c0:c0 + 512],
                        start=(kp == 0), stop=(kp == KIN // 2 - 1),
                        perf_mode=DR)
                for hl in range(2):
                    h = 2 * ct + hl
                    dst = dest[h][0:64, tsi * W + half * 512:
                                  tsi * W + half * 512 + 512]
                    src = ps_t[hl * 64:(hl + 1) * 64, :]
                    r = _cp_rot[0] = (_cp_rot[0] + 1) % 2
                    if which == 0:
                        bias = qb_sb[hl * 64:(hl + 1) * 64, ct:ct + 1]
                        if r == 0:
                            nc.scalar.activation(dst, src, Ident, bias=bias,
                                                 scale=1.0 / QKSCALE)
                        else:
                            nc.vector.tensor_scalar(
                                dst, src, 1.0 / QKSCALE, bias,
                                mybir.AluOpType.mult, mybir.AluOpType.add)
                    else:
                        if r == 0:
                            nc.scalar.activation(dst, src, Ident,
                                                 scale=1.0 / QKSCALE)
                        else:
                            nc.vector.tensor_scalar_mul(dst, src, 1.0 / QKSCALE)

            def qkproj(which, ct, tsi):
                qkproj_half(which, ct, tsi, 0)
                qkproj_half(which, ct, tsi, 1)

            # ---- v projection tile: natural layout [t, d] into vaug ----
            _sc_rot = [0]

            def vproj_tile(tt):
                ps_t = pp.tile([128, 512], f32, tag="p", bufs=2,
                               name=f"vps{tt}")
                for kt in range(KIN):
                    nc.tensor.matmul(
                        ps_t[:, 0:256],
                        xt_ap(kt, tt * 128, (tt + 1) * 128),
                        wv_sb[:, kt * 256:(kt + 1) * 256],
                        start=(kt == 0), stop=(kt == KIN - 1))
                src = ps_t[:, 0:256].rearrange("p (h c) -> p h c", c=64)
                dst = vaug[:].rearrange("p (h k) -> p h k", k=KT * 128) \
                             [:, :, tt * 128: tt * 128 + 64]
                r = _sc_rot[0] = (_sc_rot[0] + 1) % 2
                if r == 0:
                    nc.scalar.copy(dst, src)
                else:
                    nc.vector.tensor_copy(dst, src)

            # ---- o-projection (partial over this core's 256 channels) ----
            _ost_rot = [0]

            def oproj_mms(tt, half, o_ps):
                c0 = half * 512
                for ct in range(CT):
                    nc.tensor.matmul(
                        o_ps[:],
                        ypair[ct][:, tt * 128:(tt + 1) * 128],
                        wo_sb[:, ct * C + c0: ct * C + c0 + 512],
                        start=(ct == 0), stop=(ct == CT - 1))

            def oproj_half(tt, half, rotate=False):
                o_ps = pp.tile([128, 512], f32, tag="p", bufs=2,
                               name=f"ops{tt}_{half}")
                oproj_mms(tt, half, o_ps)
                c0 = half * 512
                ost = wp.tile([128, 512], bf16, tag="ost",
                              name=f"ost{tt}_{half}")
                r = _ost_rot[0] = (_ost_rot[0] + 1) % 2
                if rotate and r == 1:
                    nc.vector.tensor_copy(ost[:], o_ps[:])
                else:
                    nc.scalar.copy(ost[:], o_ps[:])
                nc.sync.dma_start(
                    out_d.ap()[tt * 128:(tt + 1) * 128, c0:c0 + 512],
                    ost[:])

            def oproj_tailpair(tt0):
                """Two token tiles' o-projection with one batched output DMA;
                copies alternate ACT/DVE (both idle at the tail)."""
                ost2 = wp.tile([128, 2 * C], bf16, tag="ost2", bufs=2,
                               name=f"ost2_{tt0}")
                for u in range(2):
                    for half in range(2):
                        o_ps = pp.tile([128, 512], f32, tag="p", bufs=2,
                                       name=f"ops{tt0 + u}_{half}")
                        oproj_mms(tt0 + u, half, o_ps)
                        dst = ost2[:, u * C + half * 512: u * C + half * 512 + 512]
                        r = _ost_rot[0] = (_ost_rot[0] + 1) % 2
                        if r == 0:
                            nc.scalar.copy(dst, o_ps[:])
                        else:
                            nc.vector.tensor_copy(dst, o_ps[:])
                nc.sync.dma_start(
                    out_d.ap()[tt0 * 128:(tt0 + 2) * 128, :]
                    .rearrange("(u p) c -> p u c", u=2),
                    ost2[:].rearrange("p (u c) -> p u c", u=2))

            ypair = [ap.tile([128, T], bf16, tag=f"ypair{ct}", name=f"ypair{ct}")
                     for ct in range(CT)]

            # ---- attention ----
            def qk_geom(h, qs, kt):
                i0 = qs * W
                off = kt * 128 - i0
                lo = max(0, off)
                hi = min(W, off + 127 + DWIN[h])
                return off, lo, hi

            def attn_steps(h, qs):
                """Generator: emits attention for (h, qs) in packed groups of
                key tiles, yielding at filler-insertion points."""
                i0 = qs * W
                n_kt = (i0 + W) // 128
                kts = [kt for kt in range(n_kt)
                       if qk_geom(h, qs, kt)[2] > qk_geom(h, qs, kt)[1]]
                # pack consecutive key tiles into single-bank score groups
                groups = []
                cur, cw = [], 0
                for kt in kts:
                    off, lo, hi = qk_geom(h, qs, kt)
                    w = hi - lo
                    if cur and cw + w > 512:
                        groups.append(cur)
                        cur, cw = [], 0
                    cur.append((kt, off, lo, hi, cw))
                    cw += w
                groups.append(cur)
                y_half = [pp.tile([128, 512], f32, tag="y", bufs=2,
                                  name=f"y{h}_{qs}_{hf}") for hf in range(2)]
                last_kt_of_half = [None, None]
                for kt in kts:
                    _, lo, hi = qk_geom(h, qs, kt)
                    for (p0, p1) in _bank_pieces(lo, hi):
                        last_kt_of_half[p0 // 512] = kt

                s_tiles = {}

                def emit_qk(gi):
                    s_ps = pp.tile([128, 512], f32, tag="s", bufs=4,
                                   name=f"s{h}_{qs}_{gi}")
                    for (kt, off, lo, hi, base) in groups[gi]:
                        nc.tensor.matmul(
                            s_ps[:, base: base + hi - lo],
                            kaug[h][:, kt * 128: kt * 128 + 128],
                            qaug[h][:, i0 + lo: i0 + hi],
                            start=True, stop=True)
                    s_tiles[gi] = s_ps

                def normalize(hf):
                    y_ps = y_half[hf]
                    recip_sb = sp.tile([64, 512], f32, tag="recip",
                                       name=f"recip{h}_{qs}_{hf}")
                    nc.vector.reciprocal(recip_sb[:], y_ps[64:128, :])
                    ct, hl = h // 2, h % 2
                    nc.vector.tensor_mul(
                        ypair[ct][hl * 64:(hl + 1) * 64,
                                  i0 + hf * 512: i0 + (hf + 1) * 512],
                        y_ps[0:64, :], recip_sb[:])

                emit_qk(0)
                if len(groups) > 1:
                    emit_qk(1)
                hf_started = [False, False]
                for gi, grp in enumerate(groups):
                    if gi + 2 < len(groups):
                        emit_qk(gi + 2)
                    yield
                    s_ps = s_tiles.pop(gi)
                    # diagonal members form a suffix of the group; merge the
                    # uniform-width run into one 3D tensor_add with the tri
                    # mask broadcast (stride-0) along the run dim
                    diag = [(lo, hi, base) for (kt, off, lo, hi, base) in grp
                            if off >= 0]
                    run = [d for d in diag if d[1] - d[0] == diag[0][1] - diag[0][0]] \
                        if diag else []
                    rest = diag[len(run):]
                    if len(run) >= 2:
                        n, w = len(run), run[0][1] - run[0][0]
                        b0 = run[0][2]
                        sview = s_ps[:, b0: b0 + n * w] \
                            .rearrange("p (n c) -> p n c", n=n)[:, :, 0:128]
                        nc.vector.tensor_add(
                            sview, sview,
                            tri_sb[:].unsqueeze(1).broadcast_to([128, n, 128]))
                    elif run:
                        rest = run + rest
                    for (lo, hi, base) in rest:
                        nc.vector.tensor_add(
                            s_ps[:, base: base + 128],
                            s_ps[:, base: base + 128], tri_sb[:])
                    cw = grp[-1][4] + grp[-1][3] - grp[-1][2]
                    pt = wp.tile([128, 512], bf16, tag="pt", bufs=4,
                                 name=f"pt{h}_{qs}_{gi}")
                    nc.scalar.activation(pt[:, 0:cw], s_ps[:, 0:cw], Exp)
                    for (kt, off, lo, hi, base) in grp:
                        for (p0, p1) in _bank_pieces(lo, hi):
                            hf = p0 // 512
                            st = not hf_started[hf]
                            hf_started[hf] = True
                            nc.tensor.matmul(
                                y_half[hf][:, p0 % 512: p0 % 512 + p1 - p0],
                                vaug[:, (h * KT + kt) * 128:
                                     (h * KT + kt) * 128 + 128],
                                pt[:, base + p0 - lo: base + p1 - lo],
                                start=st, stop=False, skip_group_check=True)
                    for hf in range(2):
                        if last_kt_of_half[hf] in [g[0] for g in grp]:
                            normalize(hf)
                    yield

            def run_attn(h, qs, fillers, density=2):
                n = 0
                for _ in attn_steps(h, qs):
                    n += 1
                    if fillers and n % density == 0:
                        fillers.pop(0)()

            # ---- phase schedule ----
            qkproj(0, 0, 0)
            qkproj(1, 0, 0)

            fill = [lambda w=w, h=h: qkproj_half(w, 1, 0, h)
                    for w in (0, 1) for h in (0, 1)]
            fill += [lambda tt=tt: vproj_tile(tt) for tt in range(8)]
            run_attn(0, 0, fill, 2)
            run_attn(1, 0, fill, 2)
            fill += [lambda w=w, h=h: qkproj_half(w, 0, 1, h)
                     for w in (0, 1) for h in (0, 1)]
            run_attn(2, 0, fill, 2)
            fill += [lambda w=w, h=h: qkproj_half(w, 1, 1, h)
                     for w in (0, 1) for h in (0, 1)]
            fill += [lambda tt=tt: vproj_tile(tt) for tt in range(8, TT)]
            run_attn(3, 0, fill, 2)
            for f in fill:
                f()
            ofill = [lambda tt=tt, hf=hf: oproj_half(tt, hf)
                     for tt in range(TT // 2) for hf in (0, 1)]
            run_attn(0, 1, ofill, 2)
            run_attn(1, 1, ofill, 2)
            run_attn(2, 1, ofill, 2)
            run_attn(3, 1, ofill, 2)
            for f in ofill:
                f()
            # batched pairs first; the final tiles go out as fine-grained
            # halves so the last DMA drains in ~1us instead of ~4
            for tt0 in range(TT // 2, TT - 2, 2):
                oproj_tailpair(tt0)
            for tt in range(TT - 2, TT):
                oproj_half(tt, 0, rotate=True)
                oproj_half(tt, 1, rotate=True)

    _dedupe_ldweights(nc)
    nc.compile()
    return nc


def _bank_pieces(a, b):
    if a < 512 and b > 512:
        return [(a, 512), (512, b)]
    return [(a, b)]


def _dedupe_ldweights(nc):
    """Remove InstLdweights whose stationary operand is identical to the
    previous PE weight load (nothing in this kernel rewrites a stationary
    tile, so the loaded weights are still valid). Waits/updates of the
    removed load are merged into the next PE instruction."""
    import concourse.mybir as mybir

    PE = mybir.EngineType.PE
    removed = 0
    for blk in nc.m.functions[0].blocks:
        prev_key = None
        pend_waits, pend_updates = [], []
        drop = []
        for inst in blk.instructions:
            if getattr(inst, "engine", None) != PE:
                continue
            tname = type(inst).__name__
            if tname == "InstLdweights":
                key = (str(inst.ins[0]), str(inst.perf_mode),
                       str(inst.tile_position), str(inst.tile_size),
                       str(inst.is_transpose))
                if key == prev_key:
                    si = inst.sync_info
                    if si is not None:
                        pend_waits.extend(list(si.on_wait))
                        pend_updates.extend(list(si.on_update))
                    drop.append(inst)
                else:
                    prev_key = key
            elif tname == "InstMatmult" and not inst.is_transpose:
                if pend_waits or pend_updates:
                    si = inst.sync_info
                    if si is None:
                        inst.sync_info = mybir.SyncInfo(
                            on_wait=pend_waits, on_update=pend_updates)
                    else:
                        si.on_wait = list(si.on_wait) + pend_waits
                        si.on_update = list(si.on_update) + pend_updates
                    pend_waits, pend_updates = [], []
            elif tname == "InstEventSemaphore":
                pass  # transparent to the weight registers
            else:
                prev_key = None  # drain/transpose/branch etc: assume clobber
        assert not (pend_waits or pend_updates), "dangling ldweights syncs"
        for inst in drop:
            blk.instructions.remove(inst)
        removed += len(drop)
    return removed


def _get_nc():
    if "nc" not in _CACHE:
        _CACHE["nc"] = _build_nc()
    return _CACHE["nc"]


def _host_inputs(x, q_w, q_b, kv_w, kv_b, o_w, o_b):
    """Build the 8 per-core input dicts."""
    x = np.asarray(x, np.float32)
    q_w = np.asarray(q_w, np.float32)
    q_b = np.asarray(q_b, np.float32)
    kv_w = np.asarray(kv_w, np.float32)

    F8 = ml_dtypes.float8_e4m3
    xt = [np.ascontiguousarray(x[b].T).astype(BF16) for b in range(B)]
    xq8 = [np.ascontiguousarray(x[b].T).astype(F8) for b in range(B)]
    j = np.arange(T, dtype=np.float32)
    ones = np.ones(T, np.float32)
    qrow = np.stack([-j, ones, ones, ones]).astype(BF16)
    tri = np.where(np.arange(128)[:, None] <= np.arange(128)[None, :],
                   np.float32(0), np.float32(NEG)).astype(np.float32)

    in_maps = []
    for c in range(NCORES):
        b, g = divmod(c, NCORES // B)
        hs = slice(g * 256, (g + 1) * 256)
        slopes = (np.arange(g * 4, g * 4 + 4, dtype=np.float32) + 1.0) / NH
        krows = np.empty((NHL * 4, T), np.float32)
        for hl in range(NHL):
            s = slopes[hl]
            krows[4 * hl + 0] = s
            krows[4 * hl + 1] = s * np.mod(j, 16)
            krows[4 * hl + 2] = s * 16 * np.mod(np.floor(j / 16), 16)
            krows[4 * hl + 3] = s * 256 * np.floor(j / 256)
        in_maps.append({
            "xt": xt[b],
            "xq8": xq8[b],
            "wq": (q_w[:, hs] * np.float32(QKSCALE / np.sqrt(HD))).astype(F8),
            "wk": (kv_w[:, hs] * np.float32(QKSCALE)).astype(F8),
            "wv": kv_w[:, C + g * 256: C + (g + 1) * 256].astype(BF16),
            "wo": np.asarray(o_w, np.float32)[hs, :].astype(BF16),
            "qb": np.ascontiguousarray(
                (q_b[hs] * np.float32(1.0 / np.sqrt(HD))).reshape(CT, 128).T),
            "qrow": qrow,
            "krows": krows.astype(BF16),
            "tri": tri,
        })
    return in_maps


def kernel(x, q_w, q_b, kv_w, kv_b, o_w, o_b):
    from concourse.bass_utils import run_bass_kernel_spmd

    nc = _get_nc()
    in_maps = _host_inputs(x, q_w, q_b, kv_w, kv_b, o_w, o_b)
    res = run_bass_kernel_spmd(nc, in_maps, core_ids=list(range(NCORES)))

    out = np.zeros((B, T, C), np.float32)
    for c in range(NCORES):
        b = c // (NCORES // B)
        out[b] += res.results[c]["o_part"].astype(np.float32)
    # analytic bias terms: v_b flows through softmax (sum=1) into o_w; o_b direct
    const_term = (np.asarray(kv_b, np.float32)[C:] @ np.asarray(o_w, np.float32)
                  + np.asarray(o_b, np.float32))
    out += const_term[None, None, :]
    return out


# revision 38
# speedup vs baseline: 1.0455x; 1.0053x over previous
"""Causal self-attention with ALiBi — Trainium2 Bass kernel, 8-core SPMD.

Problem: y = softmax(mask(q k^T / sqrt(hd) + alibi)) v, with q/kv/o projections.
B=2, T=2048, C=1024, NH=16, HD=64.

Sharding: core c handles batch b = c//4 and heads [4*(c%4), 4*(c%4)+4).
Projections are tensor-parallel over heads; each core emits a partial
o-projection (its 256 channels' contribution); the host sums the 4 partials
per batch (plus the bias terms, which are folded in analytically).

v3 design notes:
- The full ALiBi term rides inside the QK^T matmul via FOUR augmentation row
  pairs: kaug row 64 = slope (pairs with qaug row 64 = -i), and kaug rows
  65..67 = slope*(j%16), slope*16*((j//16)%16), slope*256*(j//256) pairing
  with qaug ones-rows. Each key-side value has an integer numerator <= 240 so
  it is EXACT in bf16; the fp32 psum sum reconstructs slope*j exactly.
  Query-side (-i) rounding cancels per-query in softmax. The Exp activation
  then needs no per-key-tile bias, so one exp covers a GROUP of key tiles
  packed back-to-back in one single-bank [128,512] psum tile.
- ALiBi windows tightened to theta=10 e-foldings (host-checked: ~1e-5 err).
- Score tiles are single-bank with bufs=4: the QK stream runs 2 groups ahead
  of the mask/exp/AV chain so the PE never blocks on ACT/DVE.
- k-projection bias dropped (a key-side bias cancels exactly in softmax).
- All projection psums are single-bank halves (double-buffered 8-bank psum:
  4 score + 2 y + 2 proj).
- Engine placement: exp + half the q/k copies on ACT, normalize + the other
  copies on DVE, tri-mask + v-scatter + memsets on Pool.
- Attention is a generator; projection half-chunks and v/o tiles interleave
  as PE filler between attention groups.
- o-projection rows 0..1023 DMA straight from psum to DRAM in fp32 (no
  engine copy); the tail rows 1024..2047 (engines idle by then) go through
  engine copies to bf16.
"""

import numpy as np
import ml_dtypes

B, T, C = 2, 2048, 1024
NH, HD = 16, 64
NCORES = 8
NHL = 4          # heads per core
W = 1024         # query superchunk width
NQS = T // W     # superchunks
KT = T // 128    # key tiles
CT = 2           # channel tiles for q/k projections (256 channels / 128)
KIN = C // 128   # contraction tiles for projections
TT = T // 128    # token tiles
NEG = -1.0e30
THETA = 8        # ALiBi window e-foldings
DWIN = [(THETA * 16 + hl) // (hl + 1) for hl in range(NHL)]
QKSCALE = 64.0   # fp8 weight pre-scale for the q/k projections

BF16 = ml_dtypes.bfloat16

_CACHE = {}


def _build_nc():
    import concourse.mybir as mybir
    import concourse.tile as tile
    from concourse import bacc

    f32 = mybir.dt.float32
    bf16 = mybir.dt.bfloat16
    f8 = mybir.dt.float8e4
    Exp = mybir.ActivationFunctionType.Exp
    Ident = mybir.ActivationFunctionType.Identity
    DR = mybir.MatmulPerfMode.DoubleRow

    nc = bacc.Bacc("TRN2", target_bir_lowering=False, debug=False,
                   enable_asserts=False, num_devices=NCORES)

    xt_d = nc.dram_tensor("xt", [C, T], bf16, kind="ExternalInput")
    xq8_d = nc.dram_tensor("xq8", [C, T], f8, kind="ExternalInput")
    wq_d = nc.dram_tensor("wq", [128, KIN * 256], f8, kind="ExternalInput")
    wk_d = nc.dram_tensor("wk", [128, KIN * 256], f8, kind="ExternalInput")
    wv_d = nc.dram_tensor("wv", [C, 256], bf16, kind="ExternalInput")
    wo_d = nc.dram_tensor("wo", [256, C], bf16, kind="ExternalInput")
    qb_d = nc.dram_tensor("qb", [128, CT], f32, kind="ExternalInput")
    qrow_d = nc.dram_tensor("qrow", [4, T], bf16, kind="ExternalInput")
    krows_d = nc.dram_tensor("krows", [NHL * 4, T], bf16, kind="ExternalInput")
    tri_d = nc.dram_tensor("tri", [128, 128], f32, kind="ExternalInput")
    out_d = nc.dram_tensor("o_part", [T, C], bf16, kind="ExternalOutput")

    with tile.TileContext(nc) as tc:
        with (
            tc.tile_pool(name="const", bufs=1) as cp,
            tc.tile_pool(name="aug", bufs=1) as ap,
            tc.tile_pool(name="work", bufs=10) as wp,
            tc.tile_pool(name="small", bufs=4) as sp,
            tc.tile_pool(name="ps", bufs=2, space="PSUM") as pp,
        ):
            # ---- input loads (sync/HWDGE queue, batched, in need-order) ----
            # fp8 q/k path first (cheap bytes, unblocks the PE), then the
            # bf16 x for the v-projection, wv, wo.
            wq_sb = cp.tile([128, KIN * 256], f8, tag="wq")
            wq3 = wq_sb[:].rearrange("p (k c) -> p k c", k=KIN)
            xq8_sb = cp.tile([128, KIN * T], f8, tag="xq8")
            xq3 = xq8_sb[:].rearrange("p (k t) -> p k t", k=KIN)
            xqd3 = xq8_d.ap().rearrange("(k p) t -> p k t", k=KIN)
            wk_sb = cp.tile([128, KIN * 256], f8, tag="wk")
            wk3 = wk_sb[:].rearrange("p (k c) -> p k c", k=KIN)
            # weights arrive pre-shuffled to the SBUF layout (2KB rows, one
            # descriptor-dense DMA each); xq8 loads are pair-granular
            # (DoubleRow consumes kt pairs), in first-use order
            nc.sync.dma_start(wq_sb[:], wq_d.ap()[:, :])
            nc.sync.dma_start(xq3[:, 0:2, 0:W], xqd3[:, 0:2, 0:W])
            nc.sync.dma_start(xq3[:, 2:4, 0:W], xqd3[:, 2:4, 0:W])
            nc.sync.dma_start(wk_sb[:], wk_d.ap()[:, :])
            nc.sync.dma_start(xq3[:, 4:6, 0:W], xqd3[:, 4:6, 0:W])
            nc.sync.dma_start(xq3[:, 6:8, 0:W], xqd3[:, 6:8, 0:W])
            for half in range(2):
                k0 = half * (KIN // 2)
                nc.sync.dma_start(xq3[:, k0:k0 + KIN // 2, W:T],
                                  xqd3[:, k0:k0 + KIN // 2, W:T])
            xt_sb = cp.tile([128, KIN * T], bf16, tag="xt")
            xt3 = xt_sb[:].rearrange("p (k t) -> p k t", k=KIN)
            xtd3 = xt_d.ap().rearrange("(k p) t -> p k t", k=KIN)
            for qtr in range(4):
                k0 = qtr * 2
                nc.sync.dma_start(xt3[:, k0:k0 + 2, 0:W], xtd3[:, k0:k0 + 2, 0:W])
            wv_sb = cp.tile([128, KIN * 256], bf16, tag="wv")
            nc.sync.dma_start(
                wv_sb[:].rearrange("p (k c) -> p k c", k=KIN),
                wv_d.ap().rearrange("(k p) c -> p k c", k=KIN))
            for qtr in range(4):
                k0 = qtr * 2
                nc.sync.dma_start(xt3[:, k0:k0 + 2, W:T], xtd3[:, k0:k0 + 2, W:T])
            wo_sb = cp.tile([128, CT * C], bf16, tag="wo")
            nc.sync.dma_start(
                wo_sb[:].rearrange("p (u c) -> p u c", u=CT),
                wo_d.ap().rearrange("(u p) c -> p u c", u=CT))

            def xt_ap(kt, c0, c1):
                return xt_sb[:, kt * T + c0: kt * T + c1]

            # ---- per-head augmented tensors ----
            qaug = [ap.tile([68, T], bf16, tag=f"qaug{h}", name=f"qaug{h}")
                    for h in range(NHL)]
            kaug = [ap.tile([68, T], bf16, tag=f"kaug{h}", name=f"kaug{h}")
                    for h in range(NHL)]
            # v in natural [t, d] layout, one [128, 128] block per (head, kt):
            # cols 0-63 hold v, cols 64-127 stay 1.0 (softmax denominator
            # replicated into psum rows 64-127 by the att@v matmul).
            vaug = ap.tile([128, NHL * KT * 128], bf16, tag="vaug")
            vav = vaug[:].rearrange("p (h k c) -> p h k c", h=NHL, c=128)

            # ---- small constants (Pool SWDGE queue, in need-order) ----
            qb_sb = cp.tile([128, CT], f32, tag="qb")
            nc.gpsimd.dma_start(qb_sb[:], qb_d.ap()[:, :])
            # warm the ACT exp table off the critical path
            warm = sp.tile([128, 2], bf16, tag="warm")
            nc.scalar.activation(warm[:], qb_sb[:, 0:2], Exp)
            tri_sb = cp.tile([128, 128], f32, tag="tri")
            nc.gpsimd.dma_start(tri_sb[:], tri_d.ap()[:, :])
            for h in range(2):
                nc.gpsimd.dma_start(qaug[h][64:68, :], qrow_d.ap()[:, :])
                nc.gpsimd.dma_start(kaug[h][64:68, :],
                                    krows_d.ap()[4 * h:4 * h + 4, :])
            # ones columns interleaved in AV-consumption order
            for qq in range(4):
                nc.gpsimd.memset(
                    vav[:, :, qq * (KT // 4):(qq + 1) * (KT // 4), 64:128], 1.0)
                if qq < 2:
                    h = 2 + qq
                    nc.gpsimd.dma_start(qaug[h][64:68, :], qrow_d.ap()[:, :])
                    nc.gpsimd.dma_start(kaug[h][64:68, :],
                                        krows_d.ap()[4 * h:4 * h + 4, :])

            # ---- q/k projection half-chunks ----
            _cp_rot = [0]

            # q/k projections run in fp8 with DoubleRow (2 contraction rows
            # per PE cell -> half the matmul time). Host pre-scales the fp8
            # weights by QKSCALE to stay out of subnormals; the inverse rides
            # the psum->sbuf copy.
            w3q = wq_sb[:].rearrange("p (k c) -> p k c", k=KIN)
            w3k = wk_sb[:].rearrange("p (k c) -> p k c", k=KIN)

            def qkproj_half(which, ct, tsi, half):
                w3, dest = ((w3q, qaug), (w3k, kaug))[which]
                ps_t = pp.tile([128, 512], f32, tag="p", bufs=2,
                               name=f"qkps{which}_{ct}_{tsi}_{half}")
                c0 = tsi * W + half * 512
                for kp in range(KIN // 2):
                    nc.tensor.matmul(
                        ps_t[:],
                        w3[:, 2 * kp:2 * kp + 2, ct * 128:(ct + 1) * 128],
                        xq3[:, 2 * kp:2 * kp + 2, c0:c0 + 512],
                        start=(kp == 0), stop=(kp == KIN // 2 - 1),
                        perf_mode=DR)
                for hl in range(2):
                    h = 2 * ct + hl
                    dst = dest[h][0:64, tsi * W + half * 512:
                                  tsi * W + half * 512 + 512]
                    src = ps_t[hl * 64:(hl + 1) * 64, :]
                    r = _cp_rot[0] = (_cp_rot[0] + 1) % 2
                    if which == 0:
                        bias = qb_sb[hl * 64:(hl + 1) * 64, ct:ct + 1]
                        if r == 0:
                            nc.scalar.activation(dst, src, Ident, bias=bias,
                                                 scale=1.0 / QKSCALE)
                        else:
                            nc.vector.tensor_scalar(
                                dst, src, 1.0 / QKSCALE, bias,
                                mybir.AluOpType.mult, mybir.AluOpType.add)
                    else:
                        if r == 0:
                            nc.scalar.activation(dst, src, Ident,
                                                 scale=1.0 / QKSCALE)
                        else:
                            nc.vector.tensor_scalar_mul(dst, src, 1.0 / QKSCALE)

            def qkproj(which, ct, tsi):
                qkproj_half(which, ct, tsi, 0)
                qkproj_half(which, ct, tsi, 1)

            # ---- v projection tile: natural layout [t, d] into vaug ----
            _sc_rot = [0]

            def vproj_tile(tt):
                ps_t = pp.tile([128, 512], f32, tag="p", bufs=2,
                               name=f"vps{tt}")
                for kt in range(KIN):
                    nc.tensor.matmul(
                        ps_t[:, 0:256],
                        xt_ap(kt, tt * 128, (tt + 1) * 128),
                        wv_sb[:, kt * 256:(kt + 1) * 256],
                        start=(kt == 0), stop=(kt == KIN - 1))
                src = ps_t[:, 0:256].rearrange("p (h c) -> p h c", c=64)
                dst = vaug[:].rearrange("p (h k) -> p h k", k=KT * 128) \
                             [:, :, tt * 128: tt * 128 + 64]
                r = _sc_rot[0] = (_sc_rot[0] + 1) % 2
                if r == 0:
                    nc.scalar.copy(dst, src)
                else:
                    nc.vector.tensor_copy(dst, src)

            # ---- o-projection (partial over this core's 256 channels) ----
            _ost_rot = [0]

            def oproj_mms(tt, half, o_ps):
                c0 = half * 512
                for ct in range(CT):
                    nc.tensor.matmul(
                        o_ps[:],
                        ypair[ct][:, tt * 128:(tt + 1) * 128],
                        wo_sb[:, ct * C + c0: ct * C + c0 + 512],
                        start=(ct == 0), stop=(ct == CT - 1))

            def oproj_half(tt, half, rotate=False):
                o_ps = pp.tile([128, 512], f32, tag="p", bufs=2,
                               name=f"ops{tt}_{half}")
                oproj_mms(tt, half, o_ps)
                c0 = half * 512
                ost = wp.tile([128, 512], bf16, tag="ost",
                              name=f"ost{tt}_{half}")
                r = _ost_rot[0] = (_ost_rot[0] + 1) % 2
                if rotate and r == 1:
                    nc.vector.tensor_copy(ost[:], o_ps[:])
                else:
                    nc.scalar.copy(ost[:], o_ps[:])
                nc.sync.dma_start(
                    out_d.ap()[tt * 128:(tt + 1) * 128, c0:c0 + 512],
                    ost[:])

            def oproj_tailpair(tt0):
                """Two token tiles' o-projection with one batched output DMA;
                copies alternate ACT/DVE (both idle at the tail)."""
                ost2 = wp.tile([128, 2 * C], bf16, tag="ost2", bufs=2,
                               name=f"ost2_{tt0}")
                for u in range(2):
                    for half in range(2):
                        o_ps = pp.tile([128, 512], f32, tag="p", bufs=2,
                                       name=f"ops{tt0 + u}_{half}")
                        oproj_mms(tt0 + u, half, o_ps)
                        dst = ost2[:, u * C + half * 512: u * C + half * 512 + 512]
                        r = _ost_rot[0] = (_ost_rot[0] + 1) % 2
                        if r == 0:
                            nc.scalar.copy(dst, o_ps[:])
                        else:
                            nc.vector.tensor_copy(dst, o_ps[:])
                nc.sync.dma_start(
                    out_d.ap()[tt0 * 128:(tt0 + 2) * 128, :]
                    .rearrange("(u p) c -> p u c", u=2),
                    ost2[:].rearrange("p (u c) -> p u c", u=2))

            ypair = [ap.tile([128, T], bf16, tag=f"ypair{ct}", name=f"ypair{ct}")
                     for ct in range(CT)]

            # ---- attention ----
            def qk_geom(h, qs, kt):
                i0 = qs * W
                off = kt * 128 - i0
                lo = max(0, off)
                hi = min(W, off + 127 + DWIN[h])
                return off, lo, hi

            def attn_steps(h, qs):
                """Generator: emits attention for (h, qs) in packed groups of
                key tiles, yielding at filler-insertion points."""
                i0 = qs * W
                n_kt = (i0 + W) // 128
                kts = [kt for kt in range(n_kt)
                       if qk_geom(h, qs, kt)[2] > qk_geom(h, qs, kt)[1]]
                # pack consecutive key tiles into single-bank score groups
                groups = []
                cur, cw = [], 0
                for kt in kts:
                    off, lo, hi = qk_geom(h, qs, kt)
                    w = hi - lo
                    if cur and cw + w > 512:
                        groups.append(cur)
                        cur, cw = [], 0
                    cur.append((kt, off, lo, hi, cw))
                    cw += w
                groups.append(cur)
                y_half = [pp.tile([128, 512], f32, tag="y", bufs=2,
                                  name=f"y{h}_{qs}_{hf}") for hf in range(2)]
                last_kt_of_half = [None, None]
                for kt in kts:
                    _, lo, hi = qk_geom(h, qs, kt)
                    for (p0, p1) in _bank_pieces(lo, hi):
                        last_kt_of_half[p0 // 512] = kt

                s_tiles = {}

                def emit_qk(gi):
                    s_ps = pp.tile([128, 512], f32, tag="s", bufs=4,
                                   name=f"s{h}_{qs}_{gi}")
                    for (kt, off, lo, hi, base) in groups[gi]:
                        nc.tensor.matmul(
                            s_ps[:, base: base + hi - lo],
                            kaug[h][:, kt * 128: kt * 128 + 128],
                            qaug[h][:, i0 + lo: i0 + hi],
                            start=True, stop=True)
                    s_tiles[gi] = s_ps

                def normalize(hf):
                    y_ps = y_half[hf]
                    recip_sb = sp.tile([64, 512], f32, tag="recip",
                                       name=f"recip{h}_{qs}_{hf}")
                    nc.vector.reciprocal(recip_sb[:], y_ps[64:128, :])
                    ct, hl = h // 2, h % 2
                    nc.vector.tensor_mul(
                        ypair[ct][hl * 64:(hl + 1) * 64,
                                  i0 + hf * 512: i0 + (hf + 1) * 512],
                        y_ps[0:64, :], recip_sb[:])

                emit_qk(0)
                if len(groups) > 1:
                    emit_qk(1)
                hf_started = [False, False]
                for gi, grp in enumerate(groups):
                    if gi + 2 < len(groups):
                        emit_qk(gi + 2)
                    yield
                    s_ps = s_tiles.pop(gi)
                    # diagonal members form a suffix of the group; merge the
                    # uniform-width run into one 3D tensor_add with the tri
                    # mask broadcast (stride-0) along the run dim
                    diag = [(lo, hi, base) for (kt, off, lo, hi, base) in grp
                            if off >= 0]
                    run = [d for d in diag if d[1] - d[0] == diag[0][1] - diag[0][0]] \
                        if diag else []
                    rest = diag[len(run):]
                    if len(run) >= 2:
                        n, w = len(run), run[0][1] - run[0][0]
                        b0 = run[0][2]
                        sview = s_ps[:, b0: b0 + n * w] \
                            .rearrange("p (n c) -> p n c", n=n)[:, :, 0:128]
                        nc.vector.tensor_add(
                            sview, sview,
                            tri_sb[:].unsqueeze(1).broadcast_to([128, n, 128]))
                    elif run:
                        rest = run + rest
                    for (lo, hi, base) in rest:
                        nc.vector.tensor_add(
                            s_ps[:, base: base + 128],
                            s_ps[:, base: base + 128], tri_sb[:])
                    cw = grp[-1][4] + grp[-1][3] - grp[-1][2]
                    pt = wp.tile([128, 512], bf16, tag="pt", bufs=4,
                                 name=f"pt{h}_{qs}_{gi}")
                    nc.scalar.activation(pt[:, 0:cw], s_ps[:, 0:cw], Exp)
                    for (kt, off, lo, hi, base) in grp:
                        for (p0, p1) in _bank_pieces(lo, hi):
                            hf = p0 // 512
                            st = not hf_started[hf]
                            hf_started[hf] = True
                            nc.tensor.matmul(
                                y_half[hf][:, p0 % 512: p0 % 512 + p1 - p0],
                                vaug[:, (h * KT + kt) * 128:
                                     (h * KT + kt) * 128 + 128],
                                pt[:, base + p0 - lo: base + p1 - lo],
                                start=st, stop=False, skip_group_check=True)
                    for hf in range(2):
                        if last_kt_of_half[hf] in [g[0] for g in grp]:
                            normalize(hf)
                    yield

            def run_attn(h, qs, fillers, density=2):
                n = 0
                for _ in attn_steps(h, qs):
                    n += 1
                    if fillers and n % density == 0:
                        fillers.pop(0)()

            # ---- phase schedule ----
            qkproj(0, 0, 0)
            qkproj(1, 0, 0)

            fill = [lambda w=w, h=h: qkproj_half(w, 1, 0, h)
                    for w in (0, 1) for h in (0, 1)]
            fill += [lambda tt=tt: vproj_tile(tt) for tt in range(8)]
            run_attn(0, 0, fill, 2)
            run_attn(1, 0, fill, 2)
            fill += [lambda w=w, h=h: qkproj_half(w, 0, 1, h)
                     for w in (0, 1) for h in (0, 1)]
            run_attn(2, 0, fill, 2)
            fill += [lambda w=w, h=h: qkproj_half(w, 1, 1, h)
                     for w in (0, 1) for h in (0, 1)]
            fill += [lambda tt=tt: vproj_tile(tt) for tt in range(8, TT)]
            run_attn(3, 0, fill, 2)
            for f in fill:
                f()
            ofill = [lambda tt=tt, hf=hf: oproj_half(tt, hf)
                     for tt in range(TT // 2) for hf in (0, 1)]
            run_attn(0, 1, ofill, 2)
            run_attn(1, 1, ofill, 2)
            run_attn(2, 1, ofill, 2)
            run_attn(3, 1, ofill, 2)
            for f in ofill:
                f()
            # batched pairs first; the final tiles go out as fine-grained
            # halves so the last DMA drains in ~1us instead of ~4
            for tt0 in range(TT // 2, TT - 2, 2):
                oproj_tailpair(tt0)
            for tt in range(TT - 2, TT):
                oproj_half(tt, 0, rotate=True)
                oproj_half(tt, 1, rotate=True)

    _dedupe_ldweights(nc)
    nc.compile()
    return nc


def _bank_pieces(a, b):
    if a < 512 and b > 512:
        return [(a, 512), (512, b)]
    return [(a, b)]


def _dedupe_ldweights(nc):
    """Remove InstLdweights whose stationary operand is identical to the
    previous PE weight load (nothing in this kernel rewrites a stationary
    tile, so the loaded weights are still valid). Waits/updates of the
    removed load are merged into the next PE instruction."""
    import concourse.mybir as mybir

    PE = mybir.EngineType.PE
    removed = 0
    for blk in nc.m.functions[0].blocks:
        prev_key = None
        pend_waits, pend_updates = [], []
        drop = []
        for inst in blk.instructions:
            if getattr(inst, "engine", None) != PE:
                continue
            tname = type(inst).__name__
            if tname == "InstLdweights":
                key = (str(inst.ins[0]), str(inst.perf_mode),
                       str(inst.tile_position), str(inst.tile_size),
                       str(inst.is_transpose))
                if key == prev_key:
                    si = inst.sync_info
                    if si is not None:
                        pend_waits.extend(list(si.on_wait))
                        pend_updates.extend(list(si.on_update))
                    drop.append(inst)
                else:
                    prev_key = key
            elif tname == "InstMatmult" and not inst.is_transpose:
                if pend_waits or pend_updates:
                    si = inst.sync_info
                    if si is None:
                        inst.sync_info = mybir.SyncInfo(
                            on_wait=pend_waits, on_update=pend_updates)
                    else:
                        si.on_wait = list(si.on_wait) + pend_waits
                        si.on_update = list(si.on_update) + pend_updates
                    pend_waits, pend_updates = [], []
            elif tname == "InstEventSemaphore":
                pass  # transparent to the weight registers
            else:
                prev_key = None  # drain/transpose/branch etc: assume clobber
        assert not (pend_waits or pend_updates), "dangling ldweights syncs"
        for inst in drop:
            blk.instructions.remove(inst)
        removed += len(drop)
    return removed


def _get_nc():
    if "nc" not in _CACHE:
        _CACHE["nc"] = _build_nc()
    return _CACHE["nc"]


def _host_inputs(x, q_w, q_b, kv_w, kv_b, o_w, o_b):
    """Build the 8 per-core input dicts."""
    x = np.asarray(x, np.float32)
    q_w = np.asarray(q_w, np.float32)
    q_b = np.asarray(q_b, np.float32)
    kv_w = np.asarray(kv_w, np.float32)

    F8 = ml_dtypes.float8_e4m3
    xt = [np.ascontiguousarray(x[b].T).astype(BF16) for b in range(B)]
    xq8 = [np.ascontiguousarray(x[b].T).astype(F8) for b in range(B)]
    j = np.arange(T, dtype=np.float32)
    ones = np.ones(T, np.float32)
    qrow = np.stack([-j, ones, ones, ones]).astype(BF16)
    tri = np.where(np.arange(128)[:, None] <= np.arange(128)[None, :],
                   np.float32(0), np.float32(NEG)).astype(np.float32)

    in_maps = []
    for c in range(NCORES):
        b, g = divmod(c, NCORES // B)
        hs = slice(g * 256, (g + 1) * 256)
        slopes = (np.arange(g * 4, g * 4 + 4, dtype=np.float32) + 1.0) / NH
        krows = np.empty((NHL * 4, T), np.float32)
        for hl in range(NHL):
            s = slopes[hl]
            krows[4 * hl + 0] = s
            krows[4 * hl + 1] = s * np.mod(j, 16)
            krows[4 * hl + 2] = s * 16 * np.mod(np.floor(j / 16), 16)
            krows[4 * hl + 3] = s * 256 * np.floor(j / 256)
        in_maps.append({
            "xt": xt[b],
            "xq8": xq8[b],
            "wq": np.ascontiguousarray(
                (q_w[:, hs] * np.float32(QKSCALE / np.sqrt(HD)))
                .reshape(KIN, 128, 256).transpose(1, 0, 2)
                .reshape(128, KIN * 256)).astype(F8),
            "wk": np.ascontiguousarray(
                (kv_w[:, hs] * np.float32(QKSCALE))
                .reshape(KIN, 128, 256).transpose(1, 0, 2)
                .reshape(128, KIN * 256)).astype(F8),
            "wv": kv_w[:, C + g * 256: C + (g + 1) * 256].astype(BF16),
            "wo": np.asarray(o_w, np.float32)[hs, :].astype(BF16),
            "qb": np.ascontiguousarray(
                (q_b[hs] * np.float32(1.0 / np.sqrt(HD))).reshape(CT, 128).T),
            "qrow": qrow,
            "krows": krows.astype(BF16),
            "tri": tri,
        })
    return in_maps


def kernel(x, q_w, q_b, kv_w, kv_b, o_w, o_b):
    from concourse.bass_utils import run_bass_kernel_spmd

    nc = _get_nc()
    in_maps = _host_inputs(x, q_w, q_b, kv_w, kv_b, o_w, o_b)
    res = run_bass_kernel_spmd(nc, in_maps, core_ids=list(range(NCORES)))

    out = np.zeros((B, T, C), np.float32)
    for c in range(NCORES):
        b = c // (NCORES // B)
        out[b] += res.results[c]["o_part"].astype(np.float32)
    # analytic bias terms: v_b flows through softmax (sum=1) into o_w; o_b direct
    const_term = (np.asarray(kv_b, np.float32)[C:] @ np.asarray(o_w, np.float32)
                  + np.asarray(o_b, np.float32))
    out += const_term[None, None, :]
    return out
